# revision 7
# baseline (speedup 1.0000x reference)
"""CGCNN (no BN) message-passing GNN on 8 Trainium2 NeuronCores.

Strategy (self-contained; shapes hardcoded from the problem spec):
 - Nodes are permuted on the host into 392 blocks of 128 slots, balancing
   per-block in-edge counts. Cores own 49 contiguous blocks (6272 slots).
 - Edges are owned by the core that owns their destination block; within a
   block, edges are split by source-slot half (<32768 vs >=32768) so each
   128-edge tile gathers from a single int16-indexable table view, then
   padded to a uniform (TL, TH) tile count per block so all 8 cores run one
   SPMD program.
 - Node-feature tables live in DRAM (slot-major [slots, 128] bf16); per-tile
   x[dst] / x[src] columns are pulled with DRAM-source transposed dma_gather
   (7-block chunks amortize the large fixed cost per gather call).
 - All inner-loop activations use only exp/ln/copy/square so a single ACT
   table set serves the whole kernel (a get_activation_tables patch makes
   the table-load pass pick the combined natural_log_exp set -> no
   ACT_TABLE_LOAD thrash).  msg = softplus(z2)*sigmoid(z1) is computed as
   lnb = ln(1+exp([-z1|z2])), msg = lnb_hi * exp(-lnb_lo) -- no DVE
   reciprocal.
 - LayerNorm epilogue uses ACT accum_out row-sums (mean / mean-of-squares)
   and an ACT affine (scale/bias per partition) instead of bn_aggr and the
   slow dual-scalar tensor_scalar.
 - Per-graph mean-pooling via one-hot matmuls accumulated in PSUM on the
   last layer; each core returns a reduce-scattered [32, 128] partial.
 - The tiny pooled-MLP head runs on the host in f32.
"""

import functools as _functools
import os as _os
import numpy as np
import ml_dtypes

import concourse.hw_specs as _hw_specs
import concourse.bacc as _bacc_mod

# --- activation-table patch: make exp and ln resolve to the same table set
# (natural_log_exp_and_others) so the ACT engine never reloads tables in the
# inner loop.  Only empties the shadowing set entries; ids/order unchanged.
_ORIG_ACT_TABLES = _hw_specs.get_activation_tables


@_functools.cache
def _act_tables_patched(arch):
    t = dict(_ORIG_ACT_TABLES(arch))
    for name in ("exp_and_others", "natural_log"):
        if name in t:
            t[name] = set()
    return t


_hw_specs.get_activation_tables = _act_tables_patched
_bacc_mod.get_activation_tables = _act_tables_patched

import concourse.bass as bass
import concourse.tile as tile
from concourse import bacc, mybir

BF16 = ml_dtypes.bfloat16

# Problem constants
N_NODES, N_EDGES, NODE_D, EDGE_D, EMB_D, N_GRAPHS = 50000, 800000, 128, 100, 92, 256
N_CONV, FC_D, N_FC, CUTOFF = 3, 128, 2, 6.0

LAST_RESULTS = None        # BassKernelResults of the most recent run (for tests)
LAST_RERUN_S = None        # wall seconds of a warm re-execution
LAST_RUN = None            # (runner, in_maps) of the most recent run (for tests)

N_CORES = 8
UQMAX = 6.625              # d quantization range (beyond it all gaussians ~ 0)
SLOTS = 50176              # 392 blocks * 128
BLOCKS = SLOTS // 128      # 392
NBLK = BLOCKS // N_CORES   # 49 blocks per core
CORE_SLOTS = NBLK * 128    # 6272
LO_SLOTS = 32768           # slots gatherable from the low table view
CHUNK = 7                  # blocks processed per gather chunk (49 = 7*7)

_NC_CACHE = {}


# --------------------------------------------------------------------------
# Input blob layout (shared between host packing and device program)
# --------------------------------------------------------------------------

def _layout(TL, TH, nblk=NBLK, ranks=BLOCKS, n_cores=N_CORES):
    """Byte layout of the per-core-unique and shared input blobs."""
    TPB = TL + TH
    NT = nblk * TPB
    S = NT * 128
    SLO = nblk * TL * 128
    SHI = nblk * TH * 128
    core_slots = nblk * 128

    uspec = [
        ("u", (1, S), np.int16),        # d quantized to [0, UQMAX]
        ("ixi", (16, S // 16), np.int16),
        ("ixlo", (16, SLO // 16), np.int16),
        ("ixhi", (16, SHI // 16), np.int16),
        ("zown", (16, core_slots // 16), np.int16),
        ("dstv", (128, NT), np.int8),
        ("gid", (128, nblk), np.float32),
    ]
    sspec = [
        ("ewb", (128, 128), BF16),
        ("ewf", (128, 128), np.float32),
        ("wxi", (128, N_CONV, 256), BF16),
        ("wxj", (128, N_CONV, 256), BF16),
        ("wea", (101, N_CONV, 256), BF16),
        ("lnr", (1, N_CONV * 256), np.float32),
        ("ior", (1, 256), np.float32),
        ("noffs", (101, 1), np.float32),
        ("cfs", (101, 1), np.float32),
    ]

    def place(spec):
        ents, off = {}, 0
        for name, shape, dt_ in spec:
            nbytes = int(np.prod(shape)) * np.dtype(dt_).itemsize
            ents[name] = (off, shape, dt_)
            off += -(-nbytes // 512) * 512
        return ents, off

    uents, BU = place(uspec)
    sents, BS = place(sspec)
    BU = -(-BU // 512) * 512
    align = n_cores * 512
    BS = -(-BS // align) * align
    return uents, sents, BU, BS, BS // n_cores


_NP2MY = None


def _my_dt(np_dt):
    global _NP2MY
    if _NP2MY is None:
        _NP2MY = {np.dtype(np.float32): mybir.dt.float32,
                  np.dtype(np.int16): mybir.dt.int16,
                  np.dtype(np.int8): mybir.dt.int8,
                  np.dtype(np.uint8): mybir.dt.uint8,
                  np.dtype(BF16): mybir.dt.bfloat16}
    return _NP2MY[np.dtype(np_dt)]


# --------------------------------------------------------------------------
# Device program
# --------------------------------------------------------------------------

def build_nc(TL, TH, nblk=NBLK, ranks=BLOCKS, n_cores=N_CORES,
             lo_ranks=LO_SLOTS // 128):
    """Build the SPMD Bass program. TL/TH = low/high tiles per block."""
    TPB = TL + TH                 # tiles per block
    NT = nblk * TPB               # tiles per core
    S = NT * 128                  # edge slots per core
    SLO = nblk * TL * 128
    SHI = nblk * TH * 128
    slots = ranks * 128
    core_slots = nblk * 128
    lo_slots = lo_ranks * 128
    f32, bf, i16 = mybir.dt.float32, mybir.dt.bfloat16, mybir.dt.int16
    AF = mybir.ActivationFunctionType

    nc = bacc.Bacc("TRN2", target_bir_lowering=False, debug=False,
                   num_devices=n_cores)

    # ---- external inputs: two flat blobs ---------------------------------
    uents, sents, BU, BS, BS8 = _layout(TL, TH, nblk, ranks, n_cores)
    ublob_d = nc.dram_tensor("ublob", [BU // 2], i16, kind="ExternalInput").ap()
    sblob_d = nc.dram_tensor("sblob", [BS8 // 2], i16, kind="ExternalInput").ap()
    sstage_d = nc.dram_tensor("sstage", [BS8 // 2], i16, kind="Internal").ap()
    sall_d = nc.dram_tensor("sall", [BS // 2], i16, kind="Internal",
                            addr_space="Shared").ap()

    def V(ents, base):
        def view(name):
            off, shape, dt_ = ents[name]
            nbytes = int(np.prod(shape)) * np.dtype(dt_).itemsize
            v = base[off // 2: (off + nbytes) // 2].bitcast(_my_dt(dt_))
            if len(shape) == 2:
                v = v.rearrange("(a b) -> a b", b=shape[1])
            elif len(shape) == 3:
                v = v.rearrange("(a b c) -> a b c", b=shape[1], c=shape[2])
            return v
        return view

    UV = V(uents, ublob_d)
    SV = V(sents, sall_d)
    u_d = UV("u")
    ixi_d, ixlo_d, ixhi_d = UV("ixi"), UV("ixlo"), UV("ixhi")
    zown_d, dst_d, gid_d = UV("zown"), UV("dstv"), UV("gid")
    ewb_d, ewf_d = SV("ewb"), SV("ewf")
    wxi_d, wxj_d, wea_d = SV("wxi"), SV("wxj"), SV("wea")
    lnr_d, ior_d = SV("lnr"), SV("ior")
    noffs_d, cfs_d = SV("noffs"), SV("cfs")

    # ---- internal DRAM ---------------------------------------------------
    ea_h = nc.dram_tensor("ea_h", [101, S], bf, kind="Internal").ap()
    ixi_x = nc.dram_tensor("ixi_x", [128, S // 16], i16, kind="Internal").ap()
    ixlo_x = nc.dram_tensor("ixlo_x", [128, SLO // 16], i16, kind="Internal").ap()
    ixhi_x = nc.dram_tensor("ixhi_x", [128, SHI // 16], i16, kind="Internal").ap()
    zown_x = nc.dram_tensor("zown_x", [128, core_slots // 16], i16,
                            kind="Internal").ap()
    xinit = nc.dram_tensor("xinit", [core_slots, 128], f32, kind="Internal").ap()
    xmast = [
        nc.dram_tensor(f"xmast{i}", [core_slots, 128], f32, kind="Internal").ap()
        for i in range(2)
    ]
    # per-core slot-major bf16 x tables: xown[l] = this core's x^(l) slice
    xown = [
        nc.dram_tensor(f"xown{i}", [core_slots, 128], bf, kind="Internal").ap()
        for i in range(3)
    ]
    xall = [
        nc.dram_tensor(f"xall{i}", [n_cores * core_slots, 128], bf,
                       kind="Internal", addr_space="Shared").ap()
        for i in range(3)
    ]
    gfull_d = nc.dram_tensor("gfull", [256, 128], f32, kind="Internal").ap()
    gpart_d = nc.dram_tensor("gpart", [256 // n_cores, 128], f32,
                             kind="Internal").ap()
    gsum_d = nc.dram_tensor("gsum", [256 // n_cores, 128], f32,
                            kind="ExternalOutput").ap()

    rg = [list(range(n_cores))]

    with tile.TileContext(nc) as tc:
        # reassemble the shared blob from its 8 per-core slices
        nc.sync.dma_start(sstage_d, sblob_d)
        nc.gpsimd.collective_compute(
            "AllGather", mybir.AluOpType.bypass, replica_groups=rg,
            ins=[sstage_d], outs=[sall_d])

        with tc.tile_pool(name="persist", bufs=1) as persist:
            dst_s = persist.tile([128, NT], f32)
            iota_s = persist.tile([128, 128], bf)
            io256_s = persist.tile([128, 256], bf)
            wxi_s = persist.tile([128, N_CONV, 256], bf)
            wxj_s = persist.tile([128, N_CONV, 256], bf)
            wea_s = persist.tile([101, N_CONV, 256], bf)
            g_s = persist.tile([128, N_CONV, 128], f32)
            b_s = persist.tile([128, N_CONV, 128], f32)
            gid_s = persist.tile([128, nblk], f32)
            eps_s = persist.tile([128, 1], f32)
            ones_s = persist.tile([128, 1], f32)

            nc.sync.dma_start(wxi_s[:], wxi_d)
            nc.sync.dma_start(wxj_s[:], wxj_d)
            nc.sync.dma_start(wea_s[:], wea_d)
            nc.sync.dma_start(gid_s[:], gid_d)
            nc.vector.memset(eps_s[:], 1e-5)
            nc.vector.memset(ones_s[:], 1.0)

            # ---------------- init: expansions + tables + edge features --
            with (
                tc.tile_pool(name="initp", bufs=1) as initp,
                tc.tile_pool(name="inits", bufs=3) as inits,
                tc.tile_pool(name="initps", bufs=2, space="PSUM") as initps,
            ):
                noffs_s = initp.tile([101, 1], f32, tag="noffs")
                cfs_s = initp.tile([101, 1], f32, tag="cfs")
                lnr_s = initp.tile([1, N_CONV * 256], f32, tag="lnr")
                ior_s = initp.tile([1, 256], f32, tag="ior")
                one1_s = initp.tile([1, 128], f32, tag="one1")
                on101_s = initp.tile([1, 101], f32, tag="on101")
                dstb_s = initp.tile([128, NT], mybir.dt.int8, tag="dstb")
                nc.sync.dma_start(noffs_s[:], noffs_d)
                nc.sync.dma_start(cfs_s[:], cfs_d)
                nc.sync.dma_start(lnr_s[:], lnr_d)
                nc.sync.dma_start(ior_s[:], ior_d)
                nc.vector.memset(one1_s[:], 1.0)
                nc.vector.memset(on101_s[:], 1.0)
                nc.sync.dma_start(dstb_s[:], dst_d)
                nc.vector.tensor_scalar(
                    out=dst_s[:], in0=dstb_s[:], scalar1=1.0, scalar2=None,
                    op0=mybir.AluOpType.mult)

                # replicate [16, W] index arrays into [128, W] DRAM via SBUF
                STW = 2048
                for src_d, dst_x, w in (
                    (ixi_d, ixi_x, S // 16),
                    (ixlo_d, ixlo_x, SLO // 16),
                    (ixhi_d, ixhi_x, SHI // 16),
                    (zown_d, zown_x, core_slots // 16),
                ):
                    for o in range(0, w, STW):
                        ww = min(STW, w - o)
                        st = inits.tile([16, STW], i16, tag="ix_st")
                        nc.sync.dma_start(st[:, :ww], src_d[:, o:o + ww])
                        for k in range(8):
                            nc.sync.dma_start(
                                dst_x[k * 16:(k + 1) * 16, o:o + ww],
                                st[:, :ww])

                # iota / iota256 built by broadcasting a row over partitions
                pio = initps.tile([128, 512], f32, tag="pio")
                nc.tensor.matmul(pio[:, :256], one1_s[:], ior_s[:],
                                 start=True, stop=True)
                nc.scalar.activation(io256_s[:], pio[:, :256], AF.Copy)
                nc.scalar.activation(iota_s[:], pio[:, :128], AF.Copy)
                # LN gamma/beta broadcast
                for l in range(N_CONV):
                    pln = initps.tile([128, 512], f32, tag="pln")
                    nc.tensor.matmul(pln[:, :256], one1_s[:],
                                     lnr_s[:, l * 256:(l + 1) * 256],
                                     start=True, stop=True)
                    nc.scalar.activation(g_s[:, l, :], pln[:, :128], AF.Copy)
                    nc.scalar.activation(b_s[:, l, :], pln[:, 128:256], AF.Copy)

                # z-indexed gathers build this core's x0 slice from the
                # embedding; slot-major staging -> DRAM; AllGather -> xall0
                zownt = initp.tile([128, core_slots // 16], i16, tag="zown")
                nc.sync.dma_start(zownt[:], zown_x)
                PIECE = 4096   # HW limit: big single gathers crash the device
                xb_t = initp.tile([128, nblk, 128], bf, tag="xb")
                x0_t = initp.tile([128, nblk, 128], f32, tag="x0")
                for o in range(0, core_slots, PIECE):
                    n = min(PIECE, core_slots - o)
                    nc.gpsimd.dma_gather(
                        xb_t[:, o // 128:(o + n) // 128, :], ewb_d,
                        zownt[:, o // 16:(o + n) // 16], n, n, 128,
                        transpose=False, single_packet=False)
                    nc.gpsimd.dma_gather(
                        x0_t[:, o // 128:(o + n) // 128, :], ewf_d,
                        zownt[:, o // 16:(o + n) // 16], n, n, 128,
                        transpose=False, single_packet=False)
                nc.sync.dma_start(xown[0].rearrange("(r p) c -> p r c", p=128),
                                  xb_t[:])
                nc.sync.dma_start(xinit.rearrange("(r p) c -> p r c", p=128),
                                  x0_t[:])
                nc.gpsimd.collective_compute(
                    "AllGather", mybir.AluOpType.bypass, replica_groups=rg,
                    ins=[xown[0][:]], outs=[xall[0][:]])

                # gaussian smearing: ea_k = exp(cfs_k * (d - offs_k)^2);
                # row 100 has cfs=0 -> exp(0)=1 (the conv-bias row).
                P = 512
                for off in range(0, S, P):
                    w = min(P, S - off)
                    uti = inits.tile([1, P], i16, tag="uti")
                    nc.sync.dma_start(uti[:, :w], u_d[:, off:off + w])
                    ut = inits.tile([1, P], f32, tag="ut")
                    nc.vector.tensor_scalar(
                        out=ut[:, :w], in0=uti[:, :w],
                        scalar1=float(UQMAX / 32767.0), scalar2=None,
                        op0=mybir.AluOpType.mult)
                    pe_ = initps.tile([101, P], f32, tag="pea")
                    nc.tensor.matmul(pe_[:, :w], on101_s[:], ut[:, :w],
                                     start=True, stop=True)
                    sq = inits.tile([101, P], f32, tag="sq")
                    nc.scalar.activation(sq[:, :w], pe_[:, :w], AF.Square,
                                         bias=noffs_s[:])
                    et = inits.tile([101, P], bf, tag="et")
                    nc.scalar.activation(et[:, :w], sq[:, :w], AF.Exp,
                                         scale=cfs_s[:])
                    nc.sync.dma_start(ea_h[:, off:off + w], et[:, :w])

            # ---------------- main conv layers ----------------------------
            with (
                tc.tile_pool(name="gxi", bufs=2) as gxi_p,
                tc.tile_pool(name="glo", bufs=2) as glo_p,
                tc.tile_pool(name="ghi", bufs=2) as ghi_p,
                tc.tile_pool(name="eat", bufs=3) as ea_p,
                tc.tile_pool(name="idx", bufs=2) as idx_p,
                tc.tile_pool(name="small", bufs=3) as small_p,
                tc.tile_pool(name="xio", bufs=2) as xio_p,
                tc.tile_pool(name="stats", bufs=2) as stats_p,
                tc.tile_pool(name="zc", bufs=3, space="PSUM") as zc_p,
                tc.tile_pool(name="agg", bufs=2, space="PSUM") as agg_p,
                tc.tile_pool(name="gsm", bufs=1, space="PSUM") as gsm_p,
            ):
                n_chunks = nblk // CHUNK
                assert n_chunks * CHUNK == nblk
                gsum0 = gsm_p.tile([128, 128], f32, tag="g0")
                gsum1 = gsm_p.tile([128, 128], f32, tag="g1")

                for layer in range(N_CONV):
                    last = layer == N_CONV - 1
                    xold_src = xinit if layer == 0 else xmast[layer - 1]
                    xi_tab = xown[layer]
                    xlo_tab = xall[layer][0:lo_slots, :]
                    xhi_tab = xall[layer][lo_slots:, :]

                    for ch in range(n_chunks):
                        b0 = ch * CHUNK
                        nb = CHUNK
                        n_ti = nb * TPB
                        n_tl = nb * TL
                        n_th = nb * TH

                        # ---- per-chunk index loads -------------------------
                        ixi_t = idx_p.tile([128, CHUNK * TPB * 8], i16, tag="ixi")
                        ixlo_t = idx_p.tile([128, CHUNK * TL * 8], i16, tag="ixlo")
                        ixhi_t = idx_p.tile([128, CHUNK * TH * 8], i16, tag="ixhi")
                        c0 = b0 * TPB * 8
                        nc.sync.dma_start(ixi_t[:, :n_ti * 8],
                                          ixi_x[:, c0:c0 + n_ti * 8])
                        nc.sync.dma_start(
                            ixlo_t[:, :n_tl * 8],
                            ixlo_x[:, b0 * TL * 8: b0 * TL * 8 + n_tl * 8])
                        nc.sync.dma_start(
                            ixhi_t[:, :n_th * 8],
                            ixhi_x[:, b0 * TH * 8: b0 * TH * 8 + n_th * 8])

                        # ---- gathers (DRAM-source, transposed, bf16) -------
                        xi_g = gxi_p.tile([128, 1, CHUNK * TPB * 128], bf, tag="xi")
                        lo_g = glo_p.tile([128, 1, CHUNK * TL * 128], bf, tag="lo")
                        hi_g = ghi_p.tile([128, 1, CHUNK * TH * 128], bf, tag="hi")
                        nc.gpsimd.dma_gather(
                            xi_g[:, :, :n_ti * 128], xi_tab, ixi_t[:, :n_ti * 8],
                            n_ti * 128, n_ti * 128, 128,
                            transpose=True, single_packet=False)
                        nc.gpsimd.dma_gather(
                            lo_g[:, :, :n_tl * 128], xlo_tab, ixlo_t[:, :n_tl * 8],
                            n_tl * 128, n_tl * 128, 128,
                            transpose=True, single_packet=False)
                        nc.gpsimd.dma_gather(
                            hi_g[:, :, :n_th * 128], xhi_tab, ixhi_t[:, :n_th * 8],
                            n_th * 128, n_th * 128, 128,
                            transpose=True, single_packet=False)

                        # ---- per-block compute -----------------------------
                        for bi in range(nb):
                            blk = b0 + bi
                            # edge features for this block (small DMA,
                            # alternate issuing engine to spread hw queues)
                            ea_t = ea_p.tile([101, TPB * 128], bf, tag="ea")
                            eng = (nc.sync, nc.scalar)[blk % 2]
                            eng.dma_start(
                                ea_t[:],
                                ea_h[:, blk * TPB * 128:(blk + 1) * TPB * 128])

                            agg = agg_p.tile([128, 128], f32, tag="agg")
                            for t in range(TPB):
                                is_lo = t < TL
                                xi_sl = xi_g[:, 0, (bi * TPB + t) * 128:
                                             (bi * TPB + t + 1) * 128]
                                if is_lo:
                                    xj_sl = lo_g[:, 0, (bi * TL + t) * 128:
                                                 (bi * TL + t + 1) * 128]
                                else:
                                    th = t - TL
                                    xj_sl = hi_g[:, 0, (bi * TH + th) * 128:
                                                 (bi * TH + th + 1) * 128]
                                ea_sl = ea_t[:, t * 128:(t + 1) * 128]

                                zc = zc_p.tile([128, 256], f32, tag="zc")
                                nc.tensor.matmul(zc[:], xi_sl, wxi_s[:, layer, :],
                                                 start=True, stop=False)
                                nc.tensor.matmul(zc[:], xj_sl, wxj_s[:, layer, :],
                                                 start=False, stop=False)
                                nc.tensor.matmul(zc[:], ea_sl, wea_s[:, layer, :],
                                                 start=False, stop=True)

                                sel = small_p.tile([128, 128], bf, tag="sel")
                                nc.vector.tensor_scalar(
                                    out=sel[:], in0=iota_s[:],
                                    scalar1=dst_s[:, blk * TPB + t: blk * TPB + t + 1],
                                    scalar2=None, op0=mybir.AluOpType.is_equal)

                                # zc holds [-z1 | z2] (z1-half weights
                                # sign-flipped on host).
                                # lnb = ln(1 + e^zc) = [softplus(-z1)|softplus(z2)]
                                # msg = softplus(z2) * sigmoid(z1)
                                #     = lnb_hi * exp(-lnb_lo)
                                ez = small_p.tile([128, 256], f32, tag="ez")
                                nc.scalar.activation(ez[:], zc[:], AF.Exp)
                                lnb = small_p.tile([128, 256], f32, tag="lnb")
                                nc.scalar.activation(lnb[:], ez[:], AF.Ln,
                                                     bias=ones_s[:])
                                sig = small_p.tile([128, 128], f32, tag="sig")
                                nc.scalar.activation(sig[:], lnb[:, 0:128],
                                                     AF.Exp, scale=-1.0)
                                msg = small_p.tile([128, 128], bf, tag="msg")
                                nc.vector.tensor_mul(msg[:], sig[:],
                                                     lnb[:, 128:256])

                                nc.tensor.matmul(agg[:], sel[:], msg[:],
                                                 start=(t == 0), stop=(t == TPB - 1))

                            # ---- block epilogue: LN + residual + softplus --
                            xold = xio_p.tile([128, 128], f32, tag="xold")
                            eng2 = (nc.scalar, nc.sync)[blk % 2]
                            eng2.dma_start(
                                xold[:], xold_src[blk * 128:(blk + 1) * 128, :])

                            # row sums of agg and agg^2 via ACT accumulators
                            a_s = xio_p.tile([128, 128], f32, tag="as")
                            s1 = stats_p.tile([128, 1], f32, tag="s1")
                            nc.scalar.activation(a_s[:], agg[:], AF.Copy,
                                                 accum_out=s1[:])
                            sq_t = xio_p.tile([128, 128], f32, tag="sqt")
                            s2 = stats_p.tile([128, 1], f32, tag="s2")
                            nc.scalar.activation(sq_t[:], agg[:], AF.Square,
                                                 accum_out=s2[:])
                            mean = stats_p.tile([128, 1], f32, tag="mean")
                            nc.vector.tensor_scalar(
                                out=mean[:], in0=s1[:], scalar1=1.0 / 128.0,
                                scalar2=None, op0=mybir.AluOpType.mult)
                            var = stats_p.tile([128, 1], f32, tag="var")
                            # var = s2/128 - mean^2
                            msq = stats_p.tile([128, 1], f32, tag="msq")
                            nc.vector.tensor_mul(msq[:], mean[:], mean[:])
                            nc.vector.tensor_scalar(
                                out=var[:], in0=s2[:], scalar1=1.0 / 128.0,
                                scalar2=msq[:], op0=mybir.AluOpType.mult,
                                op1=mybir.AluOpType.subtract)
                            # rstd = exp(-0.5 * ln(var + eps))
                            lnv = stats_p.tile([128, 1], f32, tag="lnv")
                            nc.scalar.activation(lnv[:], var[:], AF.Ln,
                                                 bias=eps_s[:])
                            rstd = stats_p.tile([128, 1], f32, tag="rstd")
                            nc.scalar.activation(rstd[:], lnv[:], AF.Exp,
                                                 scale=-0.5)
                            nmr = stats_p.tile([128, 1], f32, tag="nmr")
                            nc.vector.tensor_scalar(
                                out=nmr[:], in0=mean[:], scalar1=-1.0,
                                scalar2=rstd[:], op0=mybir.AluOpType.mult,
                                op1=mybir.AluOpType.mult)

                            # xn = (agg - mean) * rstd  (ACT affine)
                            xn = xio_p.tile([128, 128], f32, tag="xn")
                            nc.scalar.activation(xn[:], a_s[:], AF.Identity,
                                                 scale=rstd[:], bias=nmr[:])
                            nc.vector.tensor_mul(xn[:], xn[:], g_s[:, layer, :])
                            nc.vector.tensor_add(xn[:], xn[:], b_s[:, layer, :])
                            nc.vector.tensor_add(xn[:], xn[:], xold[:])

                            # softplus(xn) = ln(1 + e^{xn})
                            exn = xio_p.tile([128, 128], f32, tag="exn")
                            nc.scalar.activation(exn[:], xn[:], AF.Exp)
                            xnew = xio_p.tile([128, 128], f32, tag="xnew")
                            nc.scalar.activation(xnew[:], exn[:], AF.Ln,
                                                 bias=ones_s[:])
                            if not last:
                                xbf = xio_p.tile([128, 128], bf, tag="xbf")
                                nc.scalar.activation(xbf[:], xnew[:], AF.Copy)
                                eng3 = (nc.sync, nc.scalar)[(blk + 1) % 2]
                                eng3.dma_start(
                                    xown[layer + 1][blk * 128:(blk + 1) * 128, :],
                                    xbf[:])
                                eng4 = (nc.scalar, nc.sync)[(blk + 1) % 2]
                                eng4.dma_start(
                                    xmast[layer][blk * 128:(blk + 1) * 128, :],
                                    xnew[:])
                            else:
                                # pooled per-graph sums: gsum[g,:] += x[slot,:]
                                gsel = xio_p.tile([128, 256], f32, tag="gsel")
                                nc.vector.tensor_scalar(
                                    out=gsel[:], in0=io256_s[:],
                                    scalar1=gid_s[:, blk:blk + 1],
                                    scalar2=None, op0=mybir.AluOpType.is_equal)
                                nc.tensor.matmul(gsum0[:], gsel[:, 0:128],
                                                 xnew[:], start=(blk == 0),
                                                 stop=(blk == nblk - 1))
                                nc.tensor.matmul(gsum1[:], gsel[:, 128:256],
                                                 xnew[:], start=(blk == 0),
                                                 stop=(blk == nblk - 1))

                    # ---- exchange (layers 0,1): AllGather next x table ----
                    if not last:
                        nc.gpsimd.collective_compute(
                            "AllGather", mybir.AluOpType.bypass,
                            replica_groups=rg,
                            ins=[xown[layer + 1][:]], outs=[xall[layer + 1][:]])

                # ---- write pooled output ---------------------------------
                gsb = xio_p.tile([128, 256], f32, tag="gsb")
                nc.scalar.activation(gsb[:, 0:128], gsum0[:], AF.Copy)
                nc.scalar.activation(gsb[:, 128:256], gsum1[:], AF.Copy)
                nc.sync.dma_start(gfull_d[0:128, :], gsb[:, 0:128])
                nc.sync.dma_start(gfull_d[128:256, :], gsb[:, 128:256])
                nc.gpsimd.collective_compute(
                    "ReduceScatter", mybir.AluOpType.add, replica_groups=rg,
                    ins=[gfull_d], outs=[gpart_d])
                nc.sync.dma_start(gsum_d, gpart_d)

    nc.compile()
    return nc


# --------------------------------------------------------------------------
# Host preprocessing
# --------------------------------------------------------------------------

def _softplus(x):
    return np.log1p(np.exp(-np.abs(x))) + np.maximum(x, 0.0)


def _snake_slots(n, n_bins):
    """Slot offsets (bin*128 + round) for n items dealt snake-wise, in the
    order of the sorted item list."""
    idx = np.arange(n)
    r = idx // n_bins
    k = idx % n_bins
    bins = np.where(r % 2 == 0, k, n_bins - 1 - k)
    return bins * 128 + r


def _wrap16(arr):
    # [n] int16 -> [16, n/16], idx i at (i%16, i//16)
    return np.ascontiguousarray(arr.reshape(-1, 16).T)


def preprocess(z, R, edge_index, batch, embedding, emb_w, emb_b, conv_w, conv_b,
               ln_g, ln_b, n_nodes=N_NODES, n_cores=N_CORES, nblk=NBLK,
               lo_slots=LO_SLOTS, edge_d=EDGE_D, cutoff=CUTOFF):
    blocks = n_cores * nblk
    slots = blocks * 128
    core_slots = nblk * 128
    lo_blocks = lo_slots // 128
    n_edges = edge_index.shape[1]
    src = np.asarray(edge_index[0], np.int64)
    dst = np.asarray(edge_index[1], np.int64)

    # edge distances on host (smearing runs on device)
    Rf = np.asarray(R, np.float32)
    d = np.linalg.norm(Rf[src] - Rf[dst], axis=-1)  # [E] f32

    # node permutation: balance per-block in-degrees; L = orig nodes < lo_slots
    islo_e = src < lo_slots
    a = np.bincount(dst[islo_e], minlength=n_nodes)
    b = np.bincount(dst[~islo_e], minlength=n_nodes)
    w = a + b
    ordL = np.argsort(-w[:lo_slots], kind="stable")
    ordH = np.argsort(-w[lo_slots:], kind="stable") + lo_slots
    perm = np.full(n_nodes, -1, np.int64)
    perm[ordL] = _snake_slots(ordL.size, lo_blocks)
    perm[ordH] = _snake_slots(ordH.size, blocks - lo_blocks) + lo_slots
    assert perm.min() >= 0

    es, ed = perm[src], perm[dst]
    blk = ed // 128

    lo_cnt = np.bincount(blk[islo_e], minlength=blocks)
    hi_cnt = np.bincount(blk[~islo_e], minlength=blocks)
    TL = int(-(-lo_cnt.max() // 128))
    TH = int(-(-hi_cnt.max() // 128))
    TPB = TL + TH
    S = nblk * TPB * 128

    # edge slot assignment: within block, lows first then highs
    key = blk * 2 + (~islo_e).astype(np.int64)
    eorder = np.argsort(key, kind="stable")
    ks = key[eorder]
    runstart = np.r_[0, np.flatnonzero(np.diff(ks)) + 1]
    runid = np.zeros(n_edges, np.int64)
    runid[runstart[1:]] = 1
    runid = np.cumsum(runid)
    pos = np.arange(n_edges) - runstart[runid]
    eb = ks // 2
    ehalf = ks % 2
    base = eb * TPB * 128 + ehalf * (TL * 128)
    eslot_g = base + pos
    core_of = eb // nblk
    eslot = eslot_g - core_of * (nblk * TPB * 128)

    ixi = np.zeros((n_cores, S), np.int16)
    ixlo = np.zeros((n_cores, nblk * TL * 128), np.int16)
    ixhi = np.zeros((n_cores, nblk * TH * 128), np.int16)
    dstv = np.full((n_cores, nblk * TPB, 128), -1.0, np.float32)
    u = np.zeros((n_cores, 1, S), np.float32)

    e_src = es[eorder]
    e_dst = ed[eorder]
    e_lo = ehalf == 0
    d_o = d[eorder]

    for c in range(n_cores):
        m = core_of == c
        sl = eslot[m]
        # xi: dst slot local to this core's slice (0..core_slots)
        ixi[c][sl] = (e_dst[m] - c * core_slots).astype(np.int16)
        # xj
        mlo = m & e_lo
        mhi = m & ~e_lo
        slo_ = eslot[mlo]
        bb = slo_ // (TPB * 128)
        off = slo_ - bb * (TPB * 128)
        ixlo[c][bb * TL * 128 + off] = e_src[mlo].astype(np.int16)
        shi_ = eslot[mhi]
        bb = shi_ // (TPB * 128)
        off = shi_ - bb * (TPB * 128) - TL * 128
        ixhi[c][bb * TH * 128 + off] = (e_src[mhi] - lo_slots).astype(np.int16)
        # dst one-hot value, edge distances
        dstv[c].reshape(-1)[sl] = (e_dst[m] % 128).astype(np.float32)
        u[c, 0, sl] = d_o[m].astype(np.float32)

    # z tables (slot -> atom type; empty slots -> 100 which maps to a 0 row)
    zslot = np.full(slots, 100, np.int16)
    zslot[perm] = np.asarray(z, np.int16)
    # graph-id per slot (empty -> -1, excluded from pooling)
    gslot = np.full(slots, -1.0, np.float32)
    gslot[perm] = np.asarray(batch, np.float32)

    # embedding rows
    EWf = np.zeros((128, 128), np.float32)
    EWf[:100] = (np.asarray(embedding, np.float32)
                 @ np.asarray(emb_w, np.float32)
                 + np.asarray(emb_b, np.float32))
    EWb = EWf.astype(BF16)

    # conv weights; z1-half output columns sign-flipped so the device computes
    # [-z1 | z2] and can use exp/ln-only activations (one act table)
    cw = np.asarray(conv_w, np.float32).copy()
    cb = np.asarray(conv_b, np.float32).copy()
    cw[:, :, :128] *= -1.0
    cb[:, :128] *= -1.0
    wxi = np.ascontiguousarray(cw[:, :128, :].transpose(1, 0, 2)).astype(BF16)
    wxj = np.ascontiguousarray(cw[:, 128:256, :].transpose(1, 0, 2)).astype(BF16)
    wea = np.concatenate([cw[:, 256:, :], cb[:, None, :]], axis=1)
    wea = np.ascontiguousarray(wea.transpose(1, 0, 2)).astype(BF16)

    # LN gamma/beta rows
    lnr = np.concatenate(
        [np.concatenate([np.asarray(ln_g, np.float32)[l],
                         np.asarray(ln_b, np.float32)[l]])
         for l in range(cw.shape[0])])[None, :]

    # smearing: ea_k = exp(cfs_k * (d - offs_k)^2); cfs[100]=0 -> bias row 1
    offs = np.linspace(0.0, cutoff, edge_d, dtype=np.float32)
    coeff = np.float32(-0.5 / (offs[1] - offs[0]) ** 2)
    noffs = np.zeros((101, 1), np.float32)
    noffs[:edge_d, 0] = -offs
    cfs = np.zeros((101, 1), np.float32)
    cfs[:edge_d, 0] = coeff

    ior = np.arange(256, dtype=np.float32)[None, :]

    # ---- pack blobs ------------------------------------------------------
    uents, sents, BU, BS, BS8 = _layout(TL, TH, nblk, ranks=blocks,
                                        n_cores=n_cores)

    def pack(ents, arrays, nbytes):
        blob = np.zeros(nbytes // 2, np.int16)
        bv = blob.view(np.uint8)
        for name, (off, shape, dt_) in ents.items():
            a = np.ascontiguousarray(arrays[name])
            assert a.shape == tuple(shape) and a.dtype == np.dtype(dt_), \
                (name, a.shape, shape, a.dtype, dt_)
            bv[off:off + a.nbytes] = a.view(np.uint8).ravel()
        return blob

    sblob = pack(sents, {
        "ewb": EWb, "ewf": EWf,
        "wxi": wxi, "wxj": wxj, "wea": wea,
        "lnr": lnr, "ior": ior, "noffs": noffs, "cfs": cfs,
    }, BS)

    in_maps = []
    for c in range(n_cores):
        sl0 = c * core_slots
        uq = np.round(np.minimum(u[c], UQMAX) * (32767.0 / UQMAX)
                      ).astype(np.int16)
        ublob = pack(uents, {
            "u": uq,
            "ixi": _wrap16(ixi[c]),
            "ixlo": _wrap16(ixlo[c]),
            "ixhi": _wrap16(ixhi[c]),
            "zown": _wrap16(zslot[sl0:sl0 + core_slots]),
            "dstv": np.ascontiguousarray(
                dstv[c].transpose(1, 0)).astype(np.int8),
            "gid": np.ascontiguousarray(
                gslot[sl0:sl0 + core_slots].reshape(nblk, 128).T),
        }, BU)
        in_maps.append({
            "ublob": ublob,
            "sblob": sblob[c * BS8 // 2:(c + 1) * BS8 // 2],
        })
    return in_maps, TL, TH


# --------------------------------------------------------------------------
# execution: cached jitted SPMD runner (PJRT via bass2jax custom call)
# --------------------------------------------------------------------------

class _Results:
    """Minimal stand-in for BassKernelResults (test.py reads exec_time_ns)."""

    def __init__(self, results):
        self.results = results
        self.exec_time_ns = None


class _Runner:
    """Compile once, then run full numpy in_maps -> numpy outputs."""

    def __init__(self, nc, n_cores):
        import jax
        from jax.sharding import Mesh, PartitionSpec
        from jax.experimental.shard_map import shard_map
        from concourse import bass2jax

        bass2jax.install_neuronx_cc_hook()
        self.nc = nc
        self.n_cores = n_cores
        partition_name = (nc.partition_id_tensor.name
                          if nc.partition_id_tensor else None)
        in_names, out_names, out_avals, zero_outs = [], [], [], []
        for alloc in nc.m.functions[0].allocations:
            if not isinstance(alloc, mybir.MemoryLocationSet):
                continue
            name = alloc.memorylocations[0].name
            if alloc.kind == "ExternalInput":
                if name != partition_name:
                    in_names.append(name)
            elif alloc.kind == "ExternalOutput":
                shape = tuple(alloc.tensor_shape)
                dtype = mybir.dt.np(alloc.dtype)
                out_names.append(name)
                out_avals.append(jax.core.ShapedArray(shape, dtype))
                zero_outs.append(np.zeros((n_cores * shape[0], *shape[1:]),
                                          dtype))
        self.in_names = in_names
        self.out_names = out_names
        self.out_shapes = [tuple(a.shape) for a in out_avals]
        self.zero_outs = zero_outs
        n_params = len(in_names)
        all_in = in_names + out_names + (
            [partition_name] if partition_name else [])

        def _body(*args):
            operands = list(args)
            if partition_name is not None:
                operands.append(bass2jax.partition_id_tensor())
            outs = bass2jax._bass_exec_p.bind(
                *operands, out_avals=tuple(out_avals),
                in_names=tuple(all_in), out_names=tuple(out_names),
                lowering_input_output_aliases=(),
                sim_require_finite=True, sim_require_nnan=True, nc=nc)
            return tuple(outs)

        devs = jax.devices()[:n_cores]
        assert len(devs) == n_cores
        mesh = Mesh(np.asarray(devs), ("core",))
        n_outs = len(out_avals)
        self._fn = jax.jit(
            shard_map(_body, mesh=mesh,
                      in_specs=(PartitionSpec("core"),) * (n_params + n_outs),
                      out_specs=(PartitionSpec("core"),) * n_outs,
                      check_rep=False),
            donate_argnums=tuple(range(n_params, n_params + n_outs)),
            keep_unused=True)
    def run(self, in_maps):
        concat_in = [
            np.concatenate([np.asarray(m[n]) for m in in_maps], axis=0)
            for n in self.in_names]
        outs = self._fn(*concat_in, *self.zero_outs)
        n = self.n_cores
        return _Results([
            {name: np.asarray(outs[i]).reshape(n, *self.out_shapes[i])[c]
             for i, name in enumerate(self.out_names)}
            for c in range(n)])


def kernel(z, R, edge_index, batch, embedding, emb_w, emb_b, conv_w, conv_b,
           ln_g, ln_b, cfc_w, cfc_b, fc_w, fc_b, out_w, out_b):
    in_maps, TL, TH = preprocess(
        z, R, edge_index, batch, embedding, emb_w, emb_b, conv_w, conv_b,
        ln_g, ln_b)

    key = (TL, TH)
    if key not in _NC_CACHE:
        nc = build_nc(TL, TH)
        _NC_CACHE[key] = _Runner(nc, N_CORES)
    runner = _NC_CACHE[key]

    res = runner.run(in_maps)
    global LAST_RESULTS, LAST_RERUN_S, LAST_RUN
    LAST_RUN = (runner, in_maps)
    LAST_RESULTS = res
    if _os.environ.get("KERNEL_RERUN", "1") != "0":
        import time as _time
        t0 = _time.time()
        runner.run(in_maps)
        LAST_RERUN_S = _time.time() - t0

    gs = np.concatenate([res.results[c]["gsum"] for c in range(N_CORES)],
                        axis=0)  # [256, 128] fully-summed (reduce-scattered)

    batch = np.asarray(batch, np.int64)
    cnts = np.bincount(batch, minlength=N_GRAPHS).astype(np.float32)
    mol = gs / np.maximum(cnts, 1.0)[:, None]

    h = _softplus(mol @ np.asarray(cfc_w, np.float32) + np.asarray(cfc_b, np.float32))
    for l in range(np.asarray(fc_w).shape[0]):
        h = _softplus(h @ np.asarray(fc_w[l], np.float32)
                      + np.asarray(fc_b[l], np.float32))
    out = h @ np.asarray(out_w, np.float32) + np.asarray(out_b, np.float32)
    return out.astype(np.float32)


# revision 20
# speedup vs baseline: 1.5888x; 1.5888x over previous
"""CGCNN (no BN) message-passing GNN on 8 Trainium2 NeuronCores.

Strategy (self-contained; shapes hardcoded from the problem spec):
 - Nodes are permuted on the host into 392 blocks of 128 slots, balancing
   per-block in-edge counts. Cores own 49 contiguous blocks (6272 slots).
 - Edges are owned by the core that owns their destination block; within a
   block, edges are split by source-slot half (<32768 vs >=32768) so each
   128-edge tile gathers from a single int16-indexable table view, then
   padded to a uniform (TL, TH) tile count per block so all 8 cores run one
   SPMD program.
 - Node-feature tables live in DRAM (slot-major [slots, 128] bf16); per-tile
   x[dst] / x[src] columns are pulled with DRAM-source transposed dma_gather
   (7-block chunks amortize the large fixed cost per gather call).
 - All inner-loop activations use only exp/ln/copy/square so a single ACT
   table set serves the whole kernel (a get_activation_tables patch makes
   the table-load pass pick the combined natural_log_exp set -> no
   ACT_TABLE_LOAD thrash).  msg = softplus(z2)*sigmoid(z1) is computed as
   lnb = ln(1+exp([-z1|z2])), msg = lnb_hi * exp(-lnb_lo) -- no DVE
   reciprocal.
 - LayerNorm epilogue uses ACT accum_out row-sums (mean / mean-of-squares)
   and an ACT affine (scale/bias per partition) instead of bn_aggr and the
   slow dual-scalar tensor_scalar.
 - Per-graph mean-pooling via one-hot matmuls accumulated in PSUM on the
   last layer; each core returns a reduce-scattered [32, 128] partial.
 - The tiny pooled-MLP head runs on the host in f32.
"""

import functools as _functools
import os as _os
import numpy as np
import ml_dtypes

import concourse.hw_specs as _hw_specs
import concourse.bacc as _bacc_mod

# --- activation-table patch: make exp and ln resolve to the same table set
# (natural_log_exp_and_others) so the ACT engine never reloads tables in the
# inner loop.  Only empties the shadowing set entries; ids/order unchanged.
_ORIG_ACT_TABLES = _hw_specs.get_activation_tables


@_functools.cache
def _act_tables_patched(arch):
    t = dict(_ORIG_ACT_TABLES(arch))
    for name in ("exp_and_others", "natural_log"):
        if name in t:
            t[name] = set()
    return t


_hw_specs.get_activation_tables = _act_tables_patched
_bacc_mod.get_activation_tables = _act_tables_patched

import concourse.bass as bass
import concourse.tile as tile
from concourse import bacc, mybir

BF16 = ml_dtypes.bfloat16

# Problem constants
N_NODES, N_EDGES, NODE_D, EDGE_D, EMB_D, N_GRAPHS = 50000, 800000, 128, 100, 92, 256
N_CONV, FC_D, N_FC, CUTOFF = 3, 128, 2, 6.0

LAST_RESULTS = None        # BassKernelResults of the most recent run (for tests)
LAST_RERUN_S = None        # wall seconds of a warm re-execution
LAST_RUN = None            # (runner, in_maps) of the most recent run (for tests)

N_CORES = 8
UQMAX = 6.625              # d quantization range (beyond it all gaussians ~ 0)
SLOTS = 50176              # 392 blocks * 128
BLOCKS = SLOTS // 128      # 392
NBLK = BLOCKS // N_CORES   # 49 blocks per core
CORE_SLOTS = NBLK * 128    # 6272
LO_SLOTS = 32768           # slots gatherable from the low table view
CHUNK = 3                  # blocks processed per gather chunk

_NC_CACHE = {}


# --------------------------------------------------------------------------
# Input blob layout (shared between host packing and device program)
# --------------------------------------------------------------------------

def _layout(TL, TH, nblk=NBLK, ranks=BLOCKS, n_cores=N_CORES):
    """Byte layout of the per-core-unique and shared input blobs."""
    TPB = TL + TH
    NT = nblk * TPB
    S = NT * 128
    SLO = nblk * TL * 128
    SHI = nblk * TH * 128
    core_slots = nblk * 128

    uspec = [
        ("u", (1, S), np.int16),        # d quantized to [0, UQMAX]
        ("ixlo", (16, SLO // 16), np.int16),
        ("ixhi", (16, SHI // 16), np.int16),
        ("zown", (16, core_slots // 16), np.int16),
        ("dstv", (128, NT), np.int8),
        ("gid", (128, nblk), np.float32),
    ]
    sspec = [
        ("ewb", (128, 128), BF16),
        ("ewf", (128, 128), np.float32),
        ("wxi", (128, N_CONV, 256), BF16),
        ("wxj", (128, N_CONV, 256), BF16),
        ("wea", (101, N_CONV, 256), BF16),
        ("lnr", (1, N_CONV * 256), np.float32),
        ("ior", (1, 256), np.float32),
        ("noffs", (101, 1), np.float32),
        ("cfs", (101, 1), np.float32),
    ]

    def place(spec):
        ents, off = {}, 0
        for name, shape, dt_ in spec:
            nbytes = int(np.prod(shape)) * np.dtype(dt_).itemsize
            ents[name] = (off, shape, dt_)
            off += -(-nbytes // 512) * 512
        return ents, off

    uents, BU = place(uspec)
    sents, BS = place(sspec)
    BU = -(-BU // 512) * 512
    align = n_cores * 512
    BS = -(-BS // align) * align
    return uents, sents, BU, BS, BS // n_cores


_NP2MY = None


def _my_dt(np_dt):
    global _NP2MY
    if _NP2MY is None:
        _NP2MY = {np.dtype(np.float32): mybir.dt.float32,
                  np.dtype(np.int16): mybir.dt.int16,
                  np.dtype(np.int8): mybir.dt.int8,
                  np.dtype(np.uint8): mybir.dt.uint8,
                  np.dtype(BF16): mybir.dt.bfloat16}
    return _NP2MY[np.dtype(np_dt)]


# --------------------------------------------------------------------------
# Device program
# --------------------------------------------------------------------------

def build_nc(TL, TH, nblk=NBLK, ranks=BLOCKS, n_cores=N_CORES,
             lo_ranks=LO_SLOTS // 128):
    """Build the SPMD Bass program. TL/TH = low/high tiles per block."""
    TPB = TL + TH                 # tiles per block
    NT = nblk * TPB               # tiles per core
    S = NT * 128                  # edge slots per core
    SLO = nblk * TL * 128
    SHI = nblk * TH * 128
    slots = ranks * 128
    core_slots = nblk * 128
    lo_slots = lo_ranks * 128
    f32, bf, i16 = mybir.dt.float32, mybir.dt.bfloat16, mybir.dt.int16
    AF = mybir.ActivationFunctionType

    nc = bacc.Bacc("TRN2", target_bir_lowering=False, debug=False,
                   num_devices=n_cores)

    # ---- external inputs: two flat blobs ---------------------------------
    uents, sents, BU, BS, BS8 = _layout(TL, TH, nblk, ranks, n_cores)
    ublob_d = nc.dram_tensor("ublob", [BU // 2], i16, kind="ExternalInput").ap()
    sblob_d = nc.dram_tensor("sblob", [BS8 // 2], i16, kind="ExternalInput").ap()
    sstage_d = nc.dram_tensor("sstage", [BS8 // 2], i16, kind="Internal").ap()
    sall_d = nc.dram_tensor("sall", [BS // 2], i16, kind="Internal",
                            addr_space="Shared").ap()

    def V(ents, base):
        def view(name):
            off, shape, dt_ = ents[name]
            nbytes = int(np.prod(shape)) * np.dtype(dt_).itemsize
            v = base[off // 2: (off + nbytes) // 2].bitcast(_my_dt(dt_))
            if len(shape) == 2:
                v = v.rearrange("(a b) -> a b", b=shape[1])
            elif len(shape) == 3:
                v = v.rearrange("(a b c) -> a b c", b=shape[1], c=shape[2])
            return v
        return view

    UV = V(uents, ublob_d)
    SV = V(sents, sall_d)
    u_d = UV("u")
    ixlo_d, ixhi_d = UV("ixlo"), UV("ixhi")
    zown_d, dst_d, gid_d = UV("zown"), UV("dstv"), UV("gid")
    ewb_d, ewf_d = SV("ewb"), SV("ewf")
    wxi_d, wxj_d, wea_d = SV("wxi"), SV("wxj"), SV("wea")
    lnr_d, ior_d = SV("lnr"), SV("ior")
    noffs_d, cfs_d = SV("noffs"), SV("cfs")

    # ---- internal DRAM ---------------------------------------------------
    ea_h = nc.dram_tensor("ea_h", [101, S], bf, kind="Internal").ap()
    ixlo_x = nc.dram_tensor("ixlo_x", [128, SLO // 16], i16, kind="Internal").ap()
    ixhi_x = nc.dram_tensor("ixhi_x", [128, SHI // 16], i16, kind="Internal").ap()
    zown_x = nc.dram_tensor("zown_x", [128, core_slots // 16], i16,
                            kind="Internal").ap()
    xinit = nc.dram_tensor("xinit", [core_slots, 128], f32, kind="Internal").ap()
    xmast = [
        nc.dram_tensor(f"xmast{i}", [core_slots, 128], f32, kind="Internal").ap()
        for i in range(2)
    ]
    # per-core slot-major bf16 x tables: xown[l] = this core's x^(l) slice
    xown = [
        nc.dram_tensor(f"xown{i}", [core_slots, 128], bf, kind="Internal").ap()
        for i in range(3)
    ]
    xall = [
        nc.dram_tensor(f"xall{i}", [n_cores * core_slots, 128], bf,
                       kind="Internal", addr_space="Shared").ap()
        for i in range(3)
    ]
    gfull_d = nc.dram_tensor("gfull", [256, 128], f32, kind="Internal").ap()
    gpart_d = nc.dram_tensor("gpart", [256 // n_cores, 128], f32,
                             kind="Internal").ap()
    gsum_d = nc.dram_tensor("gsum", [256 // n_cores, 128], f32,
                            kind="ExternalOutput").ap()
    dbg = _os.environ.get("KERNEL_DEBUG", "0") == "1"
    if dbg:
        dx1_d = nc.dram_tensor("dx1", [core_slots, 128], bf,
                               kind="ExternalOutput").ap()
        dx2_d = nc.dram_tensor("dx2", [core_slots, 128], bf,
                               kind="ExternalOutput").ap()

    rg = [list(range(n_cores))]

    with tile.TileContext(nc) as tc:
        # reassemble the shared blob from its 8 per-core slices
        nc.sync.dma_start(sstage_d, sblob_d)
        nc.gpsimd.collective_compute(
            "AllGather", mybir.AluOpType.bypass, replica_groups=rg,
            ins=[sstage_d], outs=[sall_d])

        with tc.tile_pool(name="persist", bufs=1) as persist:
            tab_s = persist.tile([128, ranks, 128], bf)
            loc_fm = [persist.tile([128, 1, nblk * 128], bf, tag=f"fm{i}",
                                   name=f"loc_fm{i}")
                      for i in range(2)]
            ident_s = persist.tile([128, 128], f32)
            iotac_s = persist.tile([128, 1], f32)
            dst_s = persist.tile([128, NT], f32)
            iota_s = persist.tile([128, 128], bf)
            io256_s = persist.tile([128, 256], bf)
            wxi_s = persist.tile([128, N_CONV, 256], bf)
            wxj_s = persist.tile([128, N_CONV, 256], bf)
            wea_s = persist.tile([101, N_CONV, 256], bf)
            g_s = persist.tile([128, N_CONV, 128], f32)
            b_s = persist.tile([128, N_CONV, 128], f32)
            gid_s = persist.tile([128, nblk], f32)
            eps_s = persist.tile([128, 1], f32)
            ones_s = persist.tile([128, 1], f32)

            nc.sync.dma_start(wxi_s[:], wxi_d)
            nc.sync.dma_start(wxj_s[:], wxj_d)
            nc.sync.dma_start(wea_s[:], wea_d)
            nc.sync.dma_start(gid_s[:], gid_d)
            nc.vector.memset(eps_s[:], 1e-5)
            nc.vector.memset(ones_s[:], 1.0)

            # ---------------- init: expansions + tables + edge features --
            with (
                tc.tile_pool(name="initp", bufs=1) as initp,
                tc.tile_pool(name="inits", bufs=3) as inits,
                tc.tile_pool(name="initps", bufs=2, space="PSUM") as initps,
            ):
                noffs_s = initp.tile([101, 1], f32, tag="noffs")
                cfs_s = initp.tile([101, 1], f32, tag="cfs")
                lnr_s = initp.tile([1, N_CONV * 256], f32, tag="lnr")
                ior_s = initp.tile([1, 256], f32, tag="ior")
                one1_s = initp.tile([1, 128], f32, tag="one1")
                on101_s = initp.tile([1, 101], f32, tag="on101")
                dstb_s = initp.tile([128, NT], mybir.dt.int8, tag="dstb")
                nc.sync.dma_start(noffs_s[:], noffs_d)
                nc.sync.dma_start(cfs_s[:], cfs_d)
                nc.sync.dma_start(lnr_s[:], lnr_d)
                nc.sync.dma_start(ior_s[:], ior_d)
                nc.vector.memset(one1_s[:], 1.0)
                nc.vector.memset(on101_s[:], 1.0)
                nc.sync.dma_start(dstb_s[:], dst_d)
                nc.vector.tensor_scalar(
                    out=dst_s[:], in0=dstb_s[:], scalar1=1.0, scalar2=None,
                    op0=mybir.AluOpType.mult)

                # replicate [16, W] index arrays into [128, W] DRAM via SBUF
                STW = 1024
                for src_d, dst_x, w in (
                    (ixlo_d, ixlo_x, SLO // 16),
                    (ixhi_d, ixhi_x, SHI // 16),
                    (zown_d, zown_x, core_slots // 16),
                ):
                    for o in range(0, w, STW):
                        ww = min(STW, w - o)
                        st = inits.tile([16, STW], i16, tag="ix_st")
                        nc.sync.dma_start(st[:, :ww], src_d[:, o:o + ww])
                        for k in range(8):
                            eng = (nc.sync, nc.scalar)[k % 2]
                            eng.dma_start(
                                dst_x[k * 16:(k + 1) * 16, o:o + ww],
                                st[:, :ww])

                # iota / iota256 built by broadcasting a row over partitions
                pio = initps.tile([128, 512], f32, tag="pio")
                nc.tensor.matmul(pio[:, :256], one1_s[:], ior_s[:],
                                 start=True, stop=True)
                nc.scalar.activation(io256_s[:], pio[:, :256], AF.Copy)
                nc.scalar.activation(iota_s[:], pio[:, :128], AF.Copy)
                iotac_i = initp.tile([128, 1], mybir.dt.int16, tag="ioci")
                nc.gpsimd.iota(iotac_i[:], [[1, 1]], base=0,
                               channel_multiplier=1)
                nc.vector.tensor_scalar(
                    out=iotac_s[:], in0=iotac_i[:], scalar1=1.0, scalar2=None,
                    op0=mybir.AluOpType.mult)
                iotaf_s = initp.tile([128, 128], f32, tag="iotaf")
                nc.scalar.activation(iotaf_s[:], pio[:, :128], AF.Copy)
                nc.vector.tensor_scalar(
                    out=ident_s[:], in0=iotaf_s[:], scalar1=iotac_s[:],
                    scalar2=None, op0=mybir.AluOpType.is_equal)
                # LN gamma/beta broadcast
                for l in range(N_CONV):
                    pln = initps.tile([128, 512], f32, tag="pln")
                    nc.tensor.matmul(pln[:, :256], one1_s[:],
                                     lnr_s[:, l * 256:(l + 1) * 256],
                                     start=True, stop=True)
                    nc.scalar.activation(g_s[:, l, :], pln[:, :128], AF.Copy)
                    nc.scalar.activation(b_s[:, l, :], pln[:, 128:256], AF.Copy)

                # z-indexed gathers build this core's x0 slice from the
                # embedding; slot-major staging -> DRAM; AllGather -> xall0
                zownt = initp.tile([128, core_slots // 16], i16, tag="zown")
                nc.sync.dma_start(zownt[:], zown_x)
                PIECE = 4096   # HW limit: big single gathers crash the device
                xb_t = initp.tile([128, nblk, 128], bf, tag="xb")
                x0_t = initp.tile([128, nblk, 128], f32, tag="x0")
                fm0 = loc_fm[0]
                for o in range(0, core_slots, PIECE):
                    n = min(PIECE, core_slots - o)
                    nc.gpsimd.dma_gather(
                        xb_t[:, o // 128:(o + n) // 128, :], ewb_d,
                        zownt[:, o // 16:(o + n) // 16], n, n, 128,
                        transpose=False, single_packet=False)
                    nc.gpsimd.dma_gather(
                        x0_t[:, o // 128:(o + n) // 128, :], ewf_d,
                        zownt[:, o // 16:(o + n) // 16], n, n, 128,
                        transpose=False, single_packet=False)
                    # feature-major local x0 (for the per-block Y matmuls)
                    nc.gpsimd.dma_gather(
                        fm0[:, :, o:o + n], ewb_d,
                        zownt[:, o // 16:(o + n) // 16], n, n, 128,
                        transpose=True, single_packet=False)
                nc.sync.dma_start(xown[0].rearrange("(r p) c -> p r c", p=128),
                                  xb_t[:])
                nc.sync.dma_start(xinit.rearrange("(r p) c -> p r c", p=128),
                                  x0_t[:])
                nc.gpsimd.collective_compute(
                    "AllGather", mybir.AluOpType.bypass, replica_groups=rg,
                    ins=[xown[0][:]], outs=[xall[0][:]])
                xall3 = xall[0].rearrange("(r p) c -> p r c", p=128)
                HR = ranks // 2
                nc.sync.dma_start(tab_s[:, :HR, :], xall3[:, :HR, :])
                nc.scalar.dma_start(tab_s[:, HR:, :], xall3[:, HR:, :])

                # gaussian smearing: ea_k = exp(cfs_k * (d - offs_k)^2);
                # row 100 has cfs=0 -> exp(0)=1 (the conv-bias row).
                P = 512
                for off in range(0, S, P):
                    w = min(P, S - off)
                    uti = inits.tile([1, P], i16, tag="uti")
                    nc.sync.dma_start(uti[:, :w], u_d[:, off:off + w])
                    ut = inits.tile([1, P], f32, tag="ut")
                    nc.vector.tensor_scalar(
                        out=ut[:, :w], in0=uti[:, :w],
                        scalar1=float(UQMAX / 32767.0), scalar2=None,
                        op0=mybir.AluOpType.mult)
                    pe_ = initps.tile([101, P], f32, tag="pea")
                    nc.tensor.matmul(pe_[:, :w], on101_s[:], ut[:, :w],
                                     start=True, stop=True)
                    sq = inits.tile([101, P], f32, tag="sq")
                    nc.scalar.activation(sq[:, :w], pe_[:, :w], AF.Square,
                                         bias=noffs_s[:])
                    et = inits.tile([101, P], bf, tag="et")
                    nc.scalar.activation(et[:, :w], sq[:, :w], AF.Exp,
                                         scale=cfs_s[:])
                    nc.sync.dma_start(ea_h[:, off:off + w], et[:, :w])

            # ---------------- main conv layers ----------------------------
            with (
                tc.tile_pool(name="glo", bufs=2) as glo_p,
                tc.tile_pool(name="ghi", bufs=2) as ghi_p,
                tc.tile_pool(name="eat", bufs=3) as ea_p,
                tc.tile_pool(name="idx", bufs=2) as idx_p,
                tc.tile_pool(name="small", bufs=3) as small_p,
                tc.tile_pool(name="xio", bufs=2) as xio_p,
                tc.tile_pool(name="stats", bufs=2) as stats_p,
                tc.tile_pool(name="zc", bufs=2, space="PSUM") as zc_p,
                tc.tile_pool(name="agg", bufs=2, space="PSUM") as agg_p,
                tc.tile_pool(name="yp", bufs=1, space="PSUM") as y_p,
                tc.tile_pool(name="selt", bufs=1, space="PSUM") as selt_p,
                tc.tile_pool(name="gsm", bufs=1, space="PSUM") as gsm_p,
            ):
                n_chunks = (nblk + CHUNK - 1) // CHUNK
                tab_flat = tab_s.rearrange("p r c -> p (r c)")
                tab_lo_view = tab_flat[:, : lo_ranks * 128]
                tab_hi_view = tab_flat[:, lo_ranks * 128:]
                gsum0_t = gsm_p.tile([128, 128], f32, tag="g0")
                gsum1_t = gsm_p.tile([128, 128], f32, tag="g1")
                gsum0 = gsum0_t[:]
                gsum1 = gsum1_t[:]

                for layer in range(N_CONV):
                    last = layer == N_CONV - 1
                    xold_src = xinit if layer == 0 else xmast[layer - 1]
                    fm_cur = loc_fm[layer % 2]
                    fm_next = loc_fm[(layer + 1) % 2]

                    for ch in range(n_chunks):
                        b0 = ch * CHUNK
                        nb = min(CHUNK, nblk - b0)
                        n_tl = nb * TL
                        n_th = nb * TH

                        # ---- per-chunk index loads -------------------------
                        ixlo_t = idx_p.tile([128, CHUNK * TL * 8], i16, tag="ixlo")
                        ixhi_t = idx_p.tile([128, CHUNK * TH * 8], i16, tag="ixhi")
                        nc.sync.dma_start(
                            ixlo_t[:, :n_tl * 8],
                            ixlo_x[:, b0 * TL * 8: b0 * TL * 8 + n_tl * 8])
                        nc.sync.dma_start(
                            ixhi_t[:, :n_th * 8],
                            ixhi_x[:, b0 * TH * 8: b0 * TH * 8 + n_th * 8])

                        # ---- xj gathers (SBUF-source, transposed, bf16) ----
                        lo_g = glo_p.tile([128, 1, CHUNK * TL * 128], bf, tag="lo")
                        hi_g = ghi_p.tile([128, 1, CHUNK * TH * 128], bf, tag="hi")
                        nc.gpsimd.dma_gather(
                            lo_g[:, :, :n_tl * 128], tab_lo_view,
                            ixlo_t[:, :n_tl * 8],
                            n_tl * 128, n_tl * 128, 128,
                            transpose=True, sbuf_tokens_per_rank=128,
                            sbuf_free_dim_per_rank=256, single_packet=False)
                        nc.gpsimd.dma_gather(
                            hi_g[:, :, :n_th * 128], tab_hi_view,
                            ixhi_t[:, :n_th * 8],
                            n_th * 128, n_th * 128, 128,
                            transpose=True, sbuf_tokens_per_rank=128,
                            sbuf_free_dim_per_rank=256, single_packet=False)

                        # ---- per-block compute -----------------------------
                        for bi in range(nb):
                            blk = b0 + bi
                            # edge features for this block (small DMA,
                            # alternate issuing engine to spread hw queues)
                            ea_t = ea_p.tile([101, TPB * 128], bf, tag="ea")
                            eng = (nc.sync, nc.scalar)[blk % 2]
                            eng.dma_start(
                                ea_t[:],
                                ea_h[:, blk * TPB * 128:(blk + 1) * TPB * 128])

                            # Y = x_block^T @ W1  (replaces the x[dst] gather:
                            # zc1 = selT @ Y gathers rows of Y by dst slot)
                            y_ps = y_p.tile([128, 256], f32, tag="y")
                            nc.tensor.matmul(
                                y_ps[:], fm_cur[:, 0, blk * 128:(blk + 1) * 128],
                                wxi_s[:, layer, :], start=True, stop=True)
                            y_s = xio_p.tile([128, 256], bf, tag="ys")
                            nc.scalar.activation(y_s[:], y_ps[:], AF.Copy)

                            agg = agg_p.tile([128, 128], f32, tag="agg")
                            for t in range(TPB):
                                is_lo = t < TL
                                if is_lo:
                                    xj_sl = lo_g[:, 0, (bi * TL + t) * 128:
                                                 (bi * TL + t + 1) * 128]
                                else:
                                    th = t - TL
                                    xj_sl = hi_g[:, 0, (bi * TH + th) * 128:
                                                 (bi * TH + th + 1) * 128]
                                ea_sl = ea_t[:, t * 128:(t + 1) * 128]

                                dcol = dst_s[:, blk * TPB + t: blk * TPB + t + 1]
                                sel = small_p.tile([128, 128], bf, tag="sel")
                                nc.vector.tensor_scalar(
                                    out=sel[:], in0=iota_s[:],
                                    scalar1=dcol, scalar2=None,
                                    op0=mybir.AluOpType.is_equal)
                                st_ps = selt_p.tile([128, 128], f32, tag="st")
                                nc.tensor.transpose(
                                    st_ps[:], dcol.to_broadcast([128, 128]),
                                    ident_s[:])
                                selt = small_p.tile([128, 128], bf, tag="selt")
                                nc.vector.tensor_scalar(
                                    out=selt[:], in0=st_ps[:],
                                    scalar1=iotac_s[:], scalar2=None,
                                    op0=mybir.AluOpType.is_equal)

                                zc = zc_p.tile([128, 256], f32, tag="zc")
                                nc.tensor.matmul(zc[:], selt[:], y_s[:],
                                                 start=True, stop=False)
                                nc.tensor.matmul(zc[:], xj_sl, wxj_s[:, layer, :],
                                                 start=False, stop=False)
                                nc.tensor.matmul(zc[:], ea_sl, wea_s[:, layer, :],
                                                 start=False, stop=True)

                                # zc holds [-z1 | z2] (z1-half weights
                                # sign-flipped on host).
                                # lnb = ln(1 + e^zc) = [softplus(-z1)|softplus(z2)]
                                # msg = softplus(z2) * sigmoid(z1)
                                #     = lnb_hi * exp(-lnb_lo)
                                ez = small_p.tile([128, 256], f32, tag="ez")
                                nc.scalar.activation(ez[:], zc[:], AF.Exp)
                                lnb = small_p.tile([128, 256], f32, tag="lnb")
                                nc.scalar.activation(lnb[:], ez[:], AF.Ln,
                                                     bias=ones_s[:])
                                sig = small_p.tile([128, 128], f32, tag="sig")
                                nc.scalar.activation(sig[:], lnb[:, 0:128],
                                                     AF.Exp, scale=-1.0)
                                msg = small_p.tile([128, 128], bf, tag="msg")
                                nc.vector.tensor_mul(msg[:], sig[:],
                                                     lnb[:, 128:256])

                                nc.tensor.matmul(agg[:], sel[:], msg[:],
                                                 start=(t == 0), stop=(t == TPB - 1))

                            # ---- block epilogue: LN + residual + softplus --
                            xold = xio_p.tile([128, 128], f32, tag="xold")
                            eng2 = (nc.scalar, nc.sync)[blk % 2]
                            eng2.dma_start(
                                xold[:], xold_src[blk * 128:(blk + 1) * 128, :])

                            # row sums of agg and agg^2 via ACT accumulators
                            a_s = xio_p.tile([128, 128], f32, tag="as")
                            s1 = stats_p.tile([128, 1], f32, tag="s1")
                            nc.scalar.activation(a_s[:], agg[:], AF.Copy,
                                                 accum_out=s1[:])
                            sq_t = xio_p.tile([128, 128], f32, tag="sqt")
                            s2 = stats_p.tile([128, 1], f32, tag="s2")
                            nc.scalar.activation(sq_t[:], agg[:], AF.Square,
                                                 accum_out=s2[:])
                            mean = stats_p.tile([128, 1], f32, tag="mean")
                            nc.vector.tensor_scalar(
                                out=mean[:], in0=s1[:], scalar1=1.0 / 128.0,
                                scalar2=None, op0=mybir.AluOpType.mult)
                            var = stats_p.tile([128, 1], f32, tag="var")
                            # var = s2/128 - mean^2
                            msq = stats_p.tile([128, 1], f32, tag="msq")
                            nc.vector.tensor_mul(msq[:], mean[:], mean[:])
                            nc.vector.tensor_scalar(
                                out=var[:], in0=s2[:], scalar1=1.0 / 128.0,
                                scalar2=msq[:], op0=mybir.AluOpType.mult,
                                op1=mybir.AluOpType.subtract)
                            # rstd = exp(-0.5 * ln(var + eps))
                            lnv = stats_p.tile([128, 1], f32, tag="lnv")
                            nc.scalar.activation(lnv[:], var[:], AF.Ln,
                                                 bias=eps_s[:])
                            rstd = stats_p.tile([128, 1], f32, tag="rstd")
                            nc.scalar.activation(rstd[:], lnv[:], AF.Exp,
                                                 scale=-0.5)
                            nmr = stats_p.tile([128, 1], f32, tag="nmr")
                            nc.vector.tensor_scalar(
                                out=nmr[:], in0=mean[:], scalar1=-1.0,
                                scalar2=rstd[:], op0=mybir.AluOpType.mult,
                                op1=mybir.AluOpType.mult)

                            # xn = (agg - mean) * rstd  (ACT affine)
                            xn = xio_p.tile([128, 128], f32, tag="xn")
                            nc.scalar.activation(xn[:], a_s[:], AF.Identity,
                                                 scale=rstd[:], bias=nmr[:])
                            nc.vector.tensor_mul(xn[:], xn[:], g_s[:, layer, :])
                            nc.vector.tensor_add(xn[:], xn[:], b_s[:, layer, :])
                            nc.vector.tensor_add(xn[:], xn[:], xold[:])

                            # softplus(xn) = ln(1 + e^{xn})
                            exn = xio_p.tile([128, 128], f32, tag="exn")
                            nc.scalar.activation(exn[:], xn[:], AF.Exp)
                            xnew = xio_p.tile([128, 128], f32, tag="xnew")
                            nc.scalar.activation(xnew[:], exn[:], AF.Ln,
                                                 bias=ones_s[:])
                            if not last:
                                xbf = xio_p.tile([128, 128], bf, tag="xbf")
                                nc.scalar.activation(xbf[:], xnew[:], AF.Copy)
                                eng3 = (nc.sync, nc.scalar)[(blk + 1) % 2]
                                eng3.dma_start(
                                    xown[layer + 1][blk * 128:(blk + 1) * 128, :],
                                    xbf[:])
                                eng4 = (nc.scalar, nc.sync)[(blk + 1) % 2]
                                eng4.dma_start(
                                    xmast[layer][blk * 128:(blk + 1) * 128, :],
                                    xnew[:])
                                # feature-major copy for next layer's Y
                                xt_ps = selt_p.tile([128, 128], f32, tag="st")
                                nc.tensor.transpose(xt_ps[:], xnew[:], ident_s[:])
                                nc.vector.tensor_copy(
                                    fm_next[:, 0, blk * 128:(blk + 1) * 128],
                                    xt_ps[:])
                            else:
                                # pooled per-graph sums: gsum[g,:] += x[slot,:]
                                gsel = xio_p.tile([128, 256], f32, tag="gsel")
                                nc.vector.tensor_scalar(
                                    out=gsel[:], in0=io256_s[:],
                                    scalar1=gid_s[:, blk:blk + 1],
                                    scalar2=None, op0=mybir.AluOpType.is_equal)
                                nc.tensor.matmul(gsum0, gsel[:, 0:128],
                                                 xnew[:], start=(blk == 0),
                                                 stop=(blk == nblk - 1))
                                nc.tensor.matmul(gsum1, gsel[:, 128:256],
                                                 xnew[:], start=(blk == 0),
                                                 stop=(blk == nblk - 1))

                    # ---- exchange (layers 0,1): AllGather next x table ----
                    if not last:
                        nc.gpsimd.collective_compute(
                            "AllGather", mybir.AluOpType.bypass,
                            replica_groups=rg,
                            ins=[xown[layer + 1][:]], outs=[xall[layer + 1][:]])
                        xall3 = xall[layer + 1].rearrange(
                            "(r p) c -> p r c", p=128)
                        HR = ranks // 2
                        nc.sync.dma_start(tab_s[:, :HR, :], xall3[:, :HR, :])
                        nc.scalar.dma_start(tab_s[:, HR:, :], xall3[:, HR:, :])

                # ---- write pooled output ---------------------------------
                gsb = xio_p.tile([128, 256], f32, tag="gsb")
                nc.scalar.activation(gsb[:, 0:128], gsum0, AF.Copy)
                nc.scalar.activation(gsb[:, 128:256], gsum1, AF.Copy)
                nc.sync.dma_start(gfull_d[0:128, :], gsb[:, 0:128])
                nc.sync.dma_start(gfull_d[128:256, :], gsb[:, 128:256])
                nc.gpsimd.collective_compute(
                    "ReduceScatter", mybir.AluOpType.add, replica_groups=rg,
                    ins=[gfull_d], outs=[gpart_d])
                nc.sync.dma_start(gsum_d, gpart_d)
                if dbg:
                    nc.sync.dma_start(dx1_d, xown[1][:])
                    nc.sync.dma_start(dx2_d, xown[2][:])

    nc.compile()
    return nc


# --------------------------------------------------------------------------
# Host preprocessing
# --------------------------------------------------------------------------

def _softplus(x):
    return np.log1p(np.exp(-np.abs(x))) + np.maximum(x, 0.0)


def _snake_slots(n, n_bins):
    """Slot offsets (bin*128 + round) for n items dealt snake-wise, in the
    order of the sorted item list."""
    idx = np.arange(n)
    r = idx // n_bins
    k = idx % n_bins
    bins = np.where(r % 2 == 0, k, n_bins - 1 - k)
    return bins * 128 + r


def _wrap16(arr):
    # [n] int16 -> [16, n/16], idx i at (i%16, i//16)
    return np.ascontiguousarray(arr.reshape(-1, 16).T)


def preprocess(z, R, edge_index, batch, embedding, emb_w, emb_b, conv_w, conv_b,
               ln_g, ln_b, n_nodes=N_NODES, n_cores=N_CORES, nblk=NBLK,
               lo_slots=LO_SLOTS, edge_d=EDGE_D, cutoff=CUTOFF):
    blocks = n_cores * nblk
    slots = blocks * 128
    core_slots = nblk * 128
    lo_blocks = lo_slots // 128
    n_edges = edge_index.shape[1]
    src = np.asarray(edge_index[0], np.int64)
    dst = np.asarray(edge_index[1], np.int64)

    # edge distances on host (smearing runs on device)
    Rf = np.asarray(R, np.float32)
    d = np.linalg.norm(Rf[src] - Rf[dst], axis=-1)  # [E] f32

    # node permutation: balance per-block in-degrees; L = orig nodes < lo_slots
    islo_e = src < lo_slots
    a = np.bincount(dst[islo_e], minlength=n_nodes)
    b = np.bincount(dst[~islo_e], minlength=n_nodes)
    w = a + b
    ordL = np.argsort(-w[:lo_slots], kind="stable")
    ordH = np.argsort(-w[lo_slots:], kind="stable") + lo_slots
    perm = np.full(n_nodes, -1, np.int64)
    perm[ordL] = _snake_slots(ordL.size, lo_blocks)
    perm[ordH] = _snake_slots(ordH.size, blocks - lo_blocks) + lo_slots
    assert perm.min() >= 0

    es, ed = perm[src], perm[dst]
    blk = ed // 128

    lo_cnt = np.bincount(blk[islo_e], minlength=blocks)
    hi_cnt = np.bincount(blk[~islo_e], minlength=blocks)
    TL = int(-(-lo_cnt.max() // 128))
    TH = int(-(-hi_cnt.max() // 128))
    TPB = TL + TH
    S = nblk * TPB * 128

    # edge slot assignment: within block, lows first then highs
    key = blk * 2 + (~islo_e).astype(np.int64)
    eorder = np.argsort(key, kind="stable")
    ks = key[eorder]
    runstart = np.r_[0, np.flatnonzero(np.diff(ks)) + 1]
    runid = np.zeros(n_edges, np.int64)
    runid[runstart[1:]] = 1
    runid = np.cumsum(runid)
    pos = np.arange(n_edges) - runstart[runid]
    eb = ks // 2
    ehalf = ks % 2
    base = eb * TPB * 128 + ehalf * (TL * 128)
    eslot_g = base + pos
    core_of = eb // nblk
    eslot = eslot_g - core_of * (nblk * TPB * 128)

    ixlo = np.zeros((n_cores, nblk * TL * 128), np.int16)
    ixhi = np.zeros((n_cores, nblk * TH * 128), np.int16)
    dstv = np.full((n_cores, nblk * TPB, 128), -1.0, np.float32)
    u = np.zeros((n_cores, 1, S), np.float32)

    e_src = es[eorder]
    e_dst = ed[eorder]
    e_lo = ehalf == 0
    d_o = d[eorder]

    for c in range(n_cores):
        m = core_of == c
        sl = eslot[m]
        # xj
        mlo = m & e_lo
        mhi = m & ~e_lo
        slo_ = eslot[mlo]
        bb = slo_ // (TPB * 128)
        off = slo_ - bb * (TPB * 128)
        ixlo[c][bb * TL * 128 + off] = e_src[mlo].astype(np.int16)
        shi_ = eslot[mhi]
        bb = shi_ // (TPB * 128)
        off = shi_ - bb * (TPB * 128) - TL * 128
        ixhi[c][bb * TH * 128 + off] = (e_src[mhi] - lo_slots).astype(np.int16)
        # dst one-hot value, edge distances
        dstv[c].reshape(-1)[sl] = (e_dst[m] % 128).astype(np.float32)
        u[c, 0, sl] = d_o[m].astype(np.float32)

    # z tables (slot -> atom type; empty slots -> 100 which maps to a 0 row)
    zslot = np.full(slots, 100, np.int16)
    zslot[perm] = np.asarray(z, np.int16)
    # graph-id per slot (empty -> -1, excluded from pooling)
    gslot = np.full(slots, -1.0, np.float32)
    gslot[perm] = np.asarray(batch, np.float32)

    # embedding rows
    EWf = np.zeros((128, 128), np.float32)
    EWf[:100] = (np.asarray(embedding, np.float32)
                 @ np.asarray(emb_w, np.float32)
                 + np.asarray(emb_b, np.float32))
    EWb = EWf.astype(BF16)

    # conv weights; z1-half output columns sign-flipped so the device computes
    # [-z1 | z2] and can use exp/ln-only activations (one act table)
    cw = np.asarray(conv_w, np.float32).copy()
    cb = np.asarray(conv_b, np.float32).copy()
    cw[:, :, :128] *= -1.0
    cb[:, :128] *= -1.0
    wxi = np.ascontiguousarray(cw[:, :128, :].transpose(1, 0, 2)).astype(BF16)
    wxj = np.ascontiguousarray(cw[:, 128:256, :].transpose(1, 0, 2)).astype(BF16)
    wea = np.concatenate([cw[:, 256:, :], cb[:, None, :]], axis=1)
    wea = np.ascontiguousarray(wea.transpose(1, 0, 2)).astype(BF16)

    # LN gamma/beta rows
    lnr = np.concatenate(
        [np.concatenate([np.asarray(ln_g, np.float32)[l],
                         np.asarray(ln_b, np.float32)[l]])
         for l in range(cw.shape[0])])[None, :]

    # smearing: ea_k = exp(cfs_k * (d - offs_k)^2); cfs[100]=0 -> bias row 1
    offs = np.linspace(0.0, cutoff, edge_d, dtype=np.float32)
    coeff = np.float32(-0.5 / (offs[1] - offs[0]) ** 2)
    noffs = np.zeros((101, 1), np.float32)
    noffs[:edge_d, 0] = -offs
    cfs = np.zeros((101, 1), np.float32)
    cfs[:edge_d, 0] = coeff

    ior = np.arange(256, dtype=np.float32)[None, :]

    # ---- pack blobs ------------------------------------------------------
    uents, sents, BU, BS, BS8 = _layout(TL, TH, nblk, ranks=blocks,
                                        n_cores=n_cores)

    def pack(ents, arrays, nbytes):
        blob = np.zeros(nbytes // 2, np.int16)
        bv = blob.view(np.uint8)
        for name, (off, shape, dt_) in ents.items():
            a = np.ascontiguousarray(arrays[name])
            assert a.shape == tuple(shape) and a.dtype == np.dtype(dt_), \
                (name, a.shape, shape, a.dtype, dt_)
            bv[off:off + a.nbytes] = a.view(np.uint8).ravel()
        return blob

    sblob = pack(sents, {
        "ewb": EWb, "ewf": EWf,
        "wxi": wxi, "wxj": wxj, "wea": wea,
        "lnr": lnr, "ior": ior, "noffs": noffs, "cfs": cfs,
    }, BS)

    in_maps = []
    for c in range(n_cores):
        sl0 = c * core_slots
        uq = np.round(np.minimum(u[c], UQMAX) * (32767.0 / UQMAX)
                      ).astype(np.int16)
        ublob = pack(uents, {
            "u": uq,
            "ixlo": _wrap16(ixlo[c]),
            "ixhi": _wrap16(ixhi[c]),
            "zown": _wrap16(zslot[sl0:sl0 + core_slots]),
            "dstv": np.ascontiguousarray(
                dstv[c].transpose(1, 0)).astype(np.int8),
            "gid": np.ascontiguousarray(
                gslot[sl0:sl0 + core_slots].reshape(nblk, 128).T),
        }, BU)
        in_maps.append({
            "ublob": ublob,
            "sblob": sblob[c * BS8 // 2:(c + 1) * BS8 // 2],
        })
    return in_maps, TL, TH


# --------------------------------------------------------------------------
# execution: cached jitted SPMD runner (PJRT via bass2jax custom call)
# --------------------------------------------------------------------------

class _Results:
    """Minimal stand-in for BassKernelResults (test.py reads exec_time_ns)."""

    def __init__(self, results):
        self.results = results
        self.exec_time_ns = None


class _Runner:
    """Compile once, then run full numpy in_maps -> numpy outputs."""

    def __init__(self, nc, n_cores):
        import jax
        from jax.sharding import Mesh, PartitionSpec
        from jax.experimental.shard_map import shard_map
        from concourse import bass2jax

        bass2jax.install_neuronx_cc_hook()
        self.nc = nc
        self.n_cores = n_cores
        partition_name = (nc.partition_id_tensor.name
                          if nc.partition_id_tensor else None)
        in_names, out_names, out_avals, zero_outs = [], [], [], []
        for alloc in nc.m.functions[0].allocations:
            if not isinstance(alloc, mybir.MemoryLocationSet):
                continue
            name = alloc.memorylocations[0].name
            if alloc.kind == "ExternalInput":
                if name != partition_name:
                    in_names.append(name)
            elif alloc.kind == "ExternalOutput":
                shape = tuple(alloc.tensor_shape)
                dtype = mybir.dt.np(alloc.dtype)
                out_names.append(name)
                out_avals.append(jax.core.ShapedArray(shape, dtype))
                zero_outs.append(np.zeros((n_cores * shape[0], *shape[1:]),
                                          dtype))
        self.in_names = in_names
        self.out_names = out_names
        self.out_shapes = [tuple(a.shape) for a in out_avals]
        self.zero_outs = zero_outs
        n_params = len(in_names)
        all_in = in_names + out_names + (
            [partition_name] if partition_name else [])

        def _body(*args):
            operands = list(args)
            if partition_name is not None:
                operands.append(bass2jax.partition_id_tensor())
            outs = bass2jax._bass_exec_p.bind(
                *operands, out_avals=tuple(out_avals),
                in_names=tuple(all_in), out_names=tuple(out_names),
                lowering_input_output_aliases=(),
                sim_require_finite=True, sim_require_nnan=True, nc=nc)
            return tuple(outs)

        devs = jax.devices()[:n_cores]
        assert len(devs) == n_cores
        mesh = Mesh(np.asarray(devs), ("core",))
        n_outs = len(out_avals)
        self._fn = jax.jit(
            shard_map(_body, mesh=mesh,
                      in_specs=(PartitionSpec("core"),) * (n_params + n_outs),
                      out_specs=(PartitionSpec("core"),) * n_outs,
                      check_rep=False),
            donate_argnums=tuple(range(n_params, n_params + n_outs)),
            keep_unused=True)
    def run(self, in_maps):
        concat_in = [
            np.concatenate([np.asarray(m[n]) for m in in_maps], axis=0)
            for n in self.in_names]
        outs = self._fn(*concat_in, *self.zero_outs)
        n = self.n_cores
        return _Results([
            {name: np.asarray(outs[i]).reshape(n, *self.out_shapes[i])[c]
             for i, name in enumerate(self.out_names)}
            for c in range(n)])


def kernel(z, R, edge_index, batch, embedding, emb_w, emb_b, conv_w, conv_b,
           ln_g, ln_b, cfc_w, cfc_b, fc_w, fc_b, out_w, out_b):
    in_maps, TL, TH = preprocess(
        z, R, edge_index, batch, embedding, emb_w, emb_b, conv_w, conv_b,
        ln_g, ln_b)

    key = (TL, TH)
    if key not in _NC_CACHE:
        nc = build_nc(TL, TH)
        _NC_CACHE[key] = _Runner(nc, N_CORES)
    runner = _NC_CACHE[key]

    res = runner.run(in_maps)
    global LAST_RESULTS, LAST_RERUN_S, LAST_RUN
    LAST_RUN = (runner, in_maps)
    LAST_RESULTS = res
    if _os.environ.get("KERNEL_RERUN", "1") != "0":
        import time as _time
        t0 = _time.time()
        runner.run(in_maps)
        LAST_RERUN_S = _time.time() - t0

    gs = np.concatenate([res.results[c]["gsum"] for c in range(N_CORES)],
                        axis=0)  # [256, 128] fully-summed (reduce-scattered)

    batch = np.asarray(batch, np.int64)
    cnts = np.bincount(batch, minlength=N_GRAPHS).astype(np.float32)
    mol = gs / np.maximum(cnts, 1.0)[:, None]

    h = _softplus(mol @ np.asarray(cfc_w, np.float32) + np.asarray(cfc_b, np.float32))
    for l in range(np.asarray(fc_w).shape[0]):
        h = _softplus(h @ np.asarray(fc_w[l], np.float32)
                      + np.asarray(fc_b[l], np.float32))
    out = h @ np.asarray(out_w, np.float32) + np.asarray(out_b, np.float32)
    return out.astype(np.float32)


# revision 22
# speedup vs baseline: 1.8588x; 1.1699x over previous
"""CGCNN (no BN) message-passing GNN on 8 Trainium2 NeuronCores.

Strategy (self-contained; shapes hardcoded from the problem spec):
 - Nodes are permuted on the host into 392 blocks of 128 slots, balancing
   per-block in-edge counts. Cores own 49 contiguous blocks (6272 slots).
 - Edges are owned by the core that owns their destination block; within a
   block, edges are split by source-slot half (<32768 vs >=32768) so each
   128-edge tile gathers from a single int16-indexable table view, then
   padded to a uniform (TL, TH) tile count per block so all 8 cores run one
   SPMD program.
 - Node-feature tables live in DRAM (slot-major [slots, 128] bf16); per-tile
   x[dst] / x[src] columns are pulled with DRAM-source transposed dma_gather
   (7-block chunks amortize the large fixed cost per gather call).
 - All inner-loop activations use only exp/ln/copy/square so a single ACT
   table set serves the whole kernel (a get_activation_tables patch makes
   the table-load pass pick the combined natural_log_exp set -> no
   ACT_TABLE_LOAD thrash).  msg = softplus(z2)*sigmoid(z1) is computed as
   lnb = ln(1+exp([-z1|z2])), msg = lnb_hi * exp(-lnb_lo) -- no DVE
   reciprocal.
 - LayerNorm epilogue uses ACT accum_out row-sums (mean / mean-of-squares)
   and an ACT affine (scale/bias per partition) instead of bn_aggr and the
   slow dual-scalar tensor_scalar.
 - Per-graph mean-pooling via one-hot matmuls accumulated in PSUM on the
   last layer; each core returns a reduce-scattered [32, 128] partial.
 - The tiny pooled-MLP head runs on the host in f32.
"""

import functools as _functools
import os as _os
import numpy as np
import ml_dtypes

import concourse.hw_specs as _hw_specs
import concourse.bacc as _bacc_mod

# --- activation-table patch: make exp and ln resolve to the same table set
# (natural_log_exp_and_others) so the ACT engine never reloads tables in the
# inner loop.  Only empties the shadowing set entries; ids/order unchanged.
_ORIG_ACT_TABLES = _hw_specs.get_activation_tables


@_functools.cache
def _act_tables_patched(arch):
    t = dict(_ORIG_ACT_TABLES(arch))
    for name in ("exp_and_others", "natural_log"):
        if name in t:
            t[name] = set()
    return t


_hw_specs.get_activation_tables = _act_tables_patched
_bacc_mod.get_activation_tables = _act_tables_patched

import concourse.bass as bass
import concourse.tile as tile
from concourse import bacc, mybir

BF16 = ml_dtypes.bfloat16

# Problem constants
N_NODES, N_EDGES, NODE_D, EDGE_D, EMB_D, N_GRAPHS = 50000, 800000, 128, 100, 92, 256
N_CONV, FC_D, N_FC, CUTOFF = 3, 128, 2, 6.0

LAST_RESULTS = None        # BassKernelResults of the most recent run (for tests)
LAST_RERUN_S = None        # wall seconds of a warm re-execution
LAST_RUN = None            # (runner, in_maps) of the most recent run (for tests)

N_CORES = 8
UQMAX = 6.625              # d quantization range (beyond it all gaussians ~ 0)
SLOTS = 50176              # 392 blocks * 128
BLOCKS = SLOTS // 128      # 392
NBLK = BLOCKS // N_CORES   # 49 blocks per core
CORE_SLOTS = NBLK * 128    # 6272
LO_SLOTS = 32768           # slots gatherable from the low table view
CHUNK = 3                  # blocks processed per gather chunk

_NC_CACHE = {}


# --------------------------------------------------------------------------
# Input blob layout (shared between host packing and device program)
# --------------------------------------------------------------------------

def _layout(TL, TH, nblk=NBLK, ranks=BLOCKS, n_cores=N_CORES):
    """Byte layout of the per-core-unique and shared input blobs."""
    TPB = TL + TH
    NT = nblk * TPB
    S = NT * 128
    SLO = nblk * TL * 128
    SHI = nblk * TH * 128
    core_slots = nblk * 128

    uspec = [
        ("u", (1, S), np.int16),        # d quantized to [0, UQMAX]
        ("ixlo", (16, SLO // 16), np.int16),
        ("ixhi", (16, SHI // 16), np.int16),
        ("zown", (16, core_slots // 16), np.int16),
        ("dstv", (128, NT), np.int8),
        ("gid", (128, nblk), np.float32),
    ]
    sspec = [
        ("ewb", (128, 128), BF16),
        ("ewf", (128, 128), np.float32),
        ("wxi", (128, N_CONV, 256), BF16),
        ("wxj", (128, N_CONV, 256), BF16),
        ("wea", (101, N_CONV, 256), BF16),
        ("lnr", (1, N_CONV * 256), np.float32),
        ("ior", (1, 256), np.float32),
        ("noffs", (101, 1), np.float32),
        ("cfs", (101, 1), np.float32),
    ]

    def place(spec):
        ents, off = {}, 0
        for name, shape, dt_ in spec:
            nbytes = int(np.prod(shape)) * np.dtype(dt_).itemsize
            ents[name] = (off, shape, dt_)
            off += -(-nbytes // 512) * 512
        return ents, off

    uents, BU = place(uspec)
    sents, BS = place(sspec)
    BU = -(-BU // 512) * 512
    align = n_cores * 512
    BS = -(-BS // align) * align
    return uents, sents, BU, BS, BS // n_cores


_NP2MY = None


def _my_dt(np_dt):
    global _NP2MY
    if _NP2MY is None:
        _NP2MY = {np.dtype(np.float32): mybir.dt.float32,
                  np.dtype(np.int16): mybir.dt.int16,
                  np.dtype(np.int8): mybir.dt.int8,
                  np.dtype(np.uint8): mybir.dt.uint8,
                  np.dtype(BF16): mybir.dt.bfloat16}
    return _NP2MY[np.dtype(np_dt)]


# --------------------------------------------------------------------------
# Device program
# --------------------------------------------------------------------------

def build_nc(TL, TH, nblk=NBLK, ranks=BLOCKS, n_cores=N_CORES,
             lo_ranks=LO_SLOTS // 128):
    """Build the SPMD Bass program. TL/TH = low/high tiles per block."""
    TPB = TL + TH                 # tiles per block
    NT = nblk * TPB               # tiles per core
    S = NT * 128                  # edge slots per core
    SLO = nblk * TL * 128
    SHI = nblk * TH * 128
    slots = ranks * 128
    core_slots = nblk * 128
    lo_slots = lo_ranks * 128
    f32, bf, i16 = mybir.dt.float32, mybir.dt.bfloat16, mybir.dt.int16
    AF = mybir.ActivationFunctionType

    nc = bacc.Bacc("TRN2", target_bir_lowering=False, debug=False,
                   num_devices=n_cores)

    # ---- external inputs: two flat blobs ---------------------------------
    uents, sents, BU, BS, BS8 = _layout(TL, TH, nblk, ranks, n_cores)
    ublob_d = nc.dram_tensor("ublob", [BU // 2], i16, kind="ExternalInput").ap()
    sblob_d = nc.dram_tensor("sblob", [BS8 // 2], i16, kind="ExternalInput").ap()
    sstage_d = nc.dram_tensor("sstage", [BS8 // 2], i16, kind="Internal").ap()
    sall_d = nc.dram_tensor("sall", [BS // 2], i16, kind="Internal",
                            addr_space="Shared").ap()

    def V(ents, base):
        def view(name):
            off, shape, dt_ = ents[name]
            nbytes = int(np.prod(shape)) * np.dtype(dt_).itemsize
            v = base[off // 2: (off + nbytes) // 2].bitcast(_my_dt(dt_))
            if len(shape) == 2:
                v = v.rearrange("(a b) -> a b", b=shape[1])
            elif len(shape) == 3:
                v = v.rearrange("(a b c) -> a b c", b=shape[1], c=shape[2])
            return v
        return view

    UV = V(uents, ublob_d)
    SV = V(sents, sall_d)
    u_d = UV("u")
    ixlo_d, ixhi_d = UV("ixlo"), UV("ixhi")
    zown_d, dst_d, gid_d = UV("zown"), UV("dstv"), UV("gid")
    ewb_d, ewf_d = SV("ewb"), SV("ewf")
    wxi_d, wxj_d, wea_d = SV("wxi"), SV("wxj"), SV("wea")
    lnr_d, ior_d = SV("lnr"), SV("ior")
    noffs_d, cfs_d = SV("noffs"), SV("cfs")

    # ---- internal DRAM ---------------------------------------------------
    ea_h = nc.dram_tensor("ea_h", [101, S], bf, kind="Internal").ap()
    ixlo_x = nc.dram_tensor("ixlo_x", [128, SLO // 16], i16, kind="Internal").ap()
    ixhi_x = nc.dram_tensor("ixhi_x", [128, SHI // 16], i16, kind="Internal").ap()
    zown_x = nc.dram_tensor("zown_x", [128, core_slots // 16], i16,
                            kind="Internal").ap()
    xinit = nc.dram_tensor("xinit", [core_slots, 128], f32, kind="Internal").ap()
    xmast = [
        nc.dram_tensor(f"xmast{i}", [core_slots, 128], f32, kind="Internal").ap()
        for i in range(2)
    ]
    # per-core slot-major bf16 x tables: xown[l] = this core's x^(l) slice
    xown = [
        nc.dram_tensor(f"xown{i}", [core_slots, 128], bf, kind="Internal").ap()
        for i in range(3)
    ]
    xall = [
        nc.dram_tensor(f"xall{i}", [n_cores * core_slots, 128], bf,
                       kind="Internal", addr_space="Shared").ap()
        for i in range(3)
    ]
    gfull_d = nc.dram_tensor("gfull", [256, 128], f32, kind="Internal").ap()
    gpart_d = nc.dram_tensor("gpart", [256 // n_cores, 128], f32,
                             kind="Internal").ap()
    gsum_d = nc.dram_tensor("gsum", [256 // n_cores, 128], f32,
                            kind="ExternalOutput").ap()
    dbg = _os.environ.get("KERNEL_DEBUG", "0") == "1"
    if dbg:
        dx1_d = nc.dram_tensor("dx1", [core_slots, 128], bf,
                               kind="ExternalOutput").ap()
        dx2_d = nc.dram_tensor("dx2", [core_slots, 128], bf,
                               kind="ExternalOutput").ap()

    rg = [list(range(n_cores))]

    with tile.TileContext(nc) as tc:
        # reassemble the shared blob from its 8 per-core slices
        nc.sync.dma_start(sstage_d, sblob_d)
        nc.gpsimd.collective_compute(
            "AllGather", mybir.AluOpType.bypass, replica_groups=rg,
            ins=[sstage_d], outs=[sall_d])

        with tc.tile_pool(name="persist", bufs=1) as persist:
            tab_s = persist.tile([128, ranks, 128], bf)
            loc_fm = [persist.tile([128, 1, nblk * 128], bf, tag=f"fm{i}",
                                   name=f"loc_fm{i}")
                      for i in range(2)]
            ident_s = persist.tile([128, 128], f32)
            iotac_s = persist.tile([128, 1], f32)
            dst_s = persist.tile([128, NT], f32)
            iota_s = persist.tile([128, 128], bf)
            io256_s = persist.tile([128, 256], bf)
            wxi_s = persist.tile([128, N_CONV, 256], bf)
            wxj_s = persist.tile([128, N_CONV, 256], bf)
            wea_s = persist.tile([101, N_CONV, 256], bf)
            g_s = persist.tile([128, N_CONV, 128], f32)
            b_s = persist.tile([128, N_CONV, 128], f32)
            gid_s = persist.tile([128, nblk], f32)
            eps_s = persist.tile([128, 1], f32)
            ones_s = persist.tile([128, 1], f32)

            nc.sync.dma_start(wxi_s[:], wxi_d)
            nc.sync.dma_start(wxj_s[:], wxj_d)
            nc.sync.dma_start(wea_s[:], wea_d)
            nc.sync.dma_start(gid_s[:], gid_d)
            nc.vector.memset(eps_s[:], 1e-5)
            nc.vector.memset(ones_s[:], 1.0)

            # ---------------- init: expansions + tables + edge features --
            with (
                tc.tile_pool(name="initp", bufs=1) as initp,
                tc.tile_pool(name="inits", bufs=3) as inits,
                tc.tile_pool(name="initps", bufs=2, space="PSUM") as initps,
            ):
                noffs_s = initp.tile([101, 1], f32, tag="noffs")
                cfs_s = initp.tile([101, 1], f32, tag="cfs")
                lnr_s = initp.tile([1, N_CONV * 256], f32, tag="lnr")
                ior_s = initp.tile([1, 256], f32, tag="ior")
                one1_s = initp.tile([1, 128], f32, tag="one1")
                on101_s = initp.tile([1, 101], f32, tag="on101")
                dstb_s = initp.tile([128, NT], mybir.dt.int8, tag="dstb")
                nc.sync.dma_start(noffs_s[:], noffs_d)
                nc.sync.dma_start(cfs_s[:], cfs_d)
                nc.sync.dma_start(lnr_s[:], lnr_d)
                nc.sync.dma_start(ior_s[:], ior_d)
                nc.vector.memset(one1_s[:], 1.0)
                nc.vector.memset(on101_s[:], 1.0)
                nc.sync.dma_start(dstb_s[:], dst_d)
                nc.vector.tensor_scalar(
                    out=dst_s[:], in0=dstb_s[:], scalar1=1.0, scalar2=None,
                    op0=mybir.AluOpType.mult)

                # replicate [16, W] index arrays into [128, W] DRAM via SBUF
                STW = 1024
                for src_d, dst_x, w in (
                    (ixlo_d, ixlo_x, SLO // 16),
                    (ixhi_d, ixhi_x, SHI // 16),
                    (zown_d, zown_x, core_slots // 16),
                ):
                    for o in range(0, w, STW):
                        ww = min(STW, w - o)
                        st = inits.tile([16, STW], i16, tag="ix_st")
                        nc.sync.dma_start(st[:, :ww], src_d[:, o:o + ww])
                        for k in range(8):
                            eng = (nc.sync, nc.scalar)[k % 2]
                            eng.dma_start(
                                dst_x[k * 16:(k + 1) * 16, o:o + ww],
                                st[:, :ww])

                # iota / iota256 built by broadcasting a row over partitions
                pio = initps.tile([128, 512], f32, tag="pio")
                nc.tensor.matmul(pio[:, :256], one1_s[:], ior_s[:],
                                 start=True, stop=True)
                nc.scalar.activation(io256_s[:], pio[:, :256], AF.Copy)
                nc.scalar.activation(iota_s[:], pio[:, :128], AF.Copy)
                iotac_i = initp.tile([128, 1], mybir.dt.int16, tag="ioci")
                nc.gpsimd.iota(iotac_i[:], [[1, 1]], base=0,
                               channel_multiplier=1)
                nc.vector.tensor_scalar(
                    out=iotac_s[:], in0=iotac_i[:], scalar1=1.0, scalar2=None,
                    op0=mybir.AluOpType.mult)
                iotaf_s = initp.tile([128, 128], f32, tag="iotaf")
                nc.scalar.activation(iotaf_s[:], pio[:, :128], AF.Copy)
                nc.vector.tensor_scalar(
                    out=ident_s[:], in0=iotaf_s[:], scalar1=iotac_s[:],
                    scalar2=None, op0=mybir.AluOpType.is_equal)
                # LN gamma/beta broadcast
                for l in range(N_CONV):
                    pln = initps.tile([128, 512], f32, tag="pln")
                    nc.tensor.matmul(pln[:, :256], one1_s[:],
                                     lnr_s[:, l * 256:(l + 1) * 256],
                                     start=True, stop=True)
                    nc.scalar.activation(g_s[:, l, :], pln[:, :128], AF.Copy)
                    nc.scalar.activation(b_s[:, l, :], pln[:, 128:256], AF.Copy)

                # z-indexed gathers build this core's x0 slice from the
                # embedding; slot-major staging -> DRAM; AllGather -> xall0
                zownt = initp.tile([128, core_slots // 16], i16, tag="zown")
                nc.sync.dma_start(zownt[:], zown_x)
                PIECE = 4096   # HW limit: big single gathers crash the device
                xb_t = initp.tile([128, nblk, 128], bf, tag="xb")
                x0_t = initp.tile([128, nblk, 128], f32, tag="x0")
                fm0 = loc_fm[0]
                for o in range(0, core_slots, PIECE):
                    n = min(PIECE, core_slots - o)
                    nc.gpsimd.dma_gather(
                        xb_t[:, o // 128:(o + n) // 128, :], ewb_d,
                        zownt[:, o // 16:(o + n) // 16], n, n, 128,
                        transpose=False, single_packet=False)
                    nc.gpsimd.dma_gather(
                        x0_t[:, o // 128:(o + n) // 128, :], ewf_d,
                        zownt[:, o // 16:(o + n) // 16], n, n, 128,
                        transpose=False, single_packet=False)
                    # feature-major local x0 (for the per-block Y matmuls)
                    nc.gpsimd.dma_gather(
                        fm0[:, :, o:o + n], ewb_d,
                        zownt[:, o // 16:(o + n) // 16], n, n, 128,
                        transpose=True, single_packet=False)
                nc.sync.dma_start(xown[0].rearrange("(r p) c -> p r c", p=128),
                                  xb_t[:])
                nc.sync.dma_start(xinit.rearrange("(r p) c -> p r c", p=128),
                                  x0_t[:])
                nc.gpsimd.collective_compute(
                    "AllGather", mybir.AluOpType.bypass, replica_groups=rg,
                    ins=[xown[0][:]], outs=[xall[0][:]])
                xall3 = xall[0].rearrange("(r p) c -> p r c", p=128)
                HR = ranks // 2
                nc.sync.dma_start(tab_s[:, :HR, :], xall3[:, :HR, :])
                nc.scalar.dma_start(tab_s[:, HR:, :], xall3[:, HR:, :])

                # gaussian smearing: ea_k = exp(cfs_k * (d - offs_k)^2);
                # row 100 has cfs=0 -> exp(0)=1 (the conv-bias row).
                P = 512
                for off in range(0, S, P):
                    w = min(P, S - off)
                    uti = inits.tile([1, P], i16, tag="uti")
                    seng = (nc.sync, nc.scalar)[(off // P) % 2]
                    seng.dma_start(uti[:, :w], u_d[:, off:off + w])
                    ut = inits.tile([1, P], f32, tag="ut")
                    nc.vector.tensor_scalar(
                        out=ut[:, :w], in0=uti[:, :w],
                        scalar1=float(UQMAX / 32767.0), scalar2=None,
                        op0=mybir.AluOpType.mult)
                    pe_ = initps.tile([101, P], f32, tag="pea")
                    nc.tensor.matmul(pe_[:, :w], on101_s[:], ut[:, :w],
                                     start=True, stop=True)
                    sq = inits.tile([101, P], f32, tag="sq")
                    nc.scalar.activation(sq[:, :w], pe_[:, :w], AF.Square,
                                         bias=noffs_s[:])
                    et = inits.tile([101, P], bf, tag="et")
                    nc.scalar.activation(et[:, :w], sq[:, :w], AF.Exp,
                                         scale=cfs_s[:])
                    seng2 = (nc.scalar, nc.sync)[(off // P) % 2]
                    seng2.dma_start(ea_h[:, off:off + w], et[:, :w])

            # ---------------- main conv layers ----------------------------
            with (
                tc.tile_pool(name="glo", bufs=2) as glo_p,
                tc.tile_pool(name="ghi", bufs=2) as ghi_p,
                tc.tile_pool(name="eat", bufs=2) as ea_p,
                tc.tile_pool(name="idx", bufs=2) as idx_p,
                tc.tile_pool(name="small", bufs=2) as small_p,
                tc.tile_pool(name="selb", bufs=2) as selb_p,
                tc.tile_pool(name="xio", bufs=2) as xio_p,
                tc.tile_pool(name="stats", bufs=2) as stats_p,
                tc.tile_pool(name="zc", bufs=2, space="PSUM") as zc_p,
                tc.tile_pool(name="agg", bufs=2, space="PSUM") as agg_p,
                tc.tile_pool(name="yp", bufs=1, space="PSUM") as y_p,
                tc.tile_pool(name="selt", bufs=1, space="PSUM") as selt_p,
                tc.tile_pool(name="gsm", bufs=1, space="PSUM") as gsm_p,
            ):
                n_chunks = (nblk + CHUNK - 1) // CHUNK
                tab_flat = tab_s.rearrange("p r c -> p (r c)")
                tab_lo_view = tab_flat[:, : lo_ranks * 128]
                tab_hi_view = tab_flat[:, lo_ranks * 128:]
                gsum0_t = gsm_p.tile([128, 128], f32, tag="g0")
                gsum1_t = gsm_p.tile([128, 128], f32, tag="g1")
                gsum0 = gsum0_t[:]
                gsum1 = gsum1_t[:]

                for layer in range(N_CONV):
                    last = layer == N_CONV - 1
                    xold_src = xinit if layer == 0 else xmast[layer - 1]
                    fm_cur = loc_fm[layer % 2]
                    fm_next = loc_fm[(layer + 1) % 2]

                    for ch in range(n_chunks):
                        b0 = ch * CHUNK
                        nb = min(CHUNK, nblk - b0)
                        n_tl = nb * TL
                        n_th = nb * TH

                        # ---- per-chunk index loads -------------------------
                        ixlo_t = idx_p.tile([128, CHUNK * TL * 8], i16, tag="ixlo")
                        ixhi_t = idx_p.tile([128, CHUNK * TH * 8], i16, tag="ixhi")
                        nc.sync.dma_start(
                            ixlo_t[:, :n_tl * 8],
                            ixlo_x[:, b0 * TL * 8: b0 * TL * 8 + n_tl * 8])
                        nc.sync.dma_start(
                            ixhi_t[:, :n_th * 8],
                            ixhi_x[:, b0 * TH * 8: b0 * TH * 8 + n_th * 8])

                        # ---- xj gathers (SBUF-source, transposed, bf16) ----
                        lo_g = glo_p.tile([128, 1, CHUNK * TL * 128], bf, tag="lo")
                        hi_g = ghi_p.tile([128, 1, CHUNK * TH * 128], bf, tag="hi")
                        nc.gpsimd.dma_gather(
                            lo_g[:, :, :n_tl * 128], tab_lo_view,
                            ixlo_t[:, :n_tl * 8],
                            n_tl * 128, n_tl * 128, 128,
                            transpose=True, sbuf_tokens_per_rank=128,
                            sbuf_free_dim_per_rank=256, single_packet=False)
                        nc.gpsimd.dma_gather(
                            hi_g[:, :, :n_th * 128], tab_hi_view,
                            ixhi_t[:, :n_th * 8],
                            n_th * 128, n_th * 128, 128,
                            transpose=True, sbuf_tokens_per_rank=128,
                            sbuf_free_dim_per_rank=256, single_packet=False)

                        # ---- per-block compute -----------------------------
                        for bi in range(nb):
                            blk = b0 + bi
                            # edge features for this block (small DMA,
                            # alternate issuing engine to spread hw queues)
                            ea_t = ea_p.tile([101, TPB * 128], bf, tag="ea")
                            eng = (nc.sync, nc.scalar)[blk % 2]
                            eng.dma_start(
                                ea_t[:],
                                ea_h[:, blk * TPB * 128:(blk + 1) * TPB * 128])

                            # Y = x_block^T @ W1  (replaces the x[dst] gather:
                            # zc1 = selT @ Y gathers rows of Y by dst slot)
                            y_ps = y_p.tile([128, 256], f32, tag="y")
                            nc.tensor.matmul(
                                y_ps[:], fm_cur[:, 0, blk * 128:(blk + 1) * 128],
                                wxi_s[:, layer, :], start=True, stop=True)
                            y_s = xio_p.tile([128, 256], bf, tag="ys")
                            nc.scalar.activation(y_s[:], y_ps[:], AF.Copy)

                            # one-hot dst matrices for all tiles of the
                            # block in a single batched is_equal
                            sel_blk = selb_p.tile([128, TPB, 128], bf, tag="sb")
                            dsl = dst_s[:, blk * TPB:(blk + 1) * TPB]
                            nc.vector.tensor_tensor(
                                out=sel_blk[:],
                                in0=iota_s[:, None, :].broadcast_to(
                                    [128, TPB, 128]),
                                in1=dsl[:, :, None].broadcast_to(
                                    [128, TPB, 128]),
                                op=mybir.AluOpType.is_equal)

                            agg = agg_p.tile([128, 128], f32, tag="agg")
                            pairs = [(t, min(t + 1, TPB - 1)) if t + 1 < TPB
                                     else (t, t) for t in range(0, TPB, 2)]
                            for t0 in range(0, TPB, 2):
                                npair = min(2, TPB - t0)
                                # transposed one-hots (two closed PE groups
                                # into slices of one PSUM bank)
                                st_ps = selt_p.tile([128, 2, 128], f32, tag="st")
                                for k in range(npair):
                                    dcol = dst_s[:, blk * TPB + t0 + k:
                                                 blk * TPB + t0 + k + 1]
                                    nc.tensor.transpose(
                                        st_ps[:, k, :],
                                        dcol.to_broadcast([128, 128]),
                                        ident_s[:])
                                selt = small_p.tile([128, 2, 128], bf, tag="selt")
                                nc.vector.tensor_scalar(
                                    out=selt[:, :npair, :],
                                    in0=st_ps[:, :npair, :],
                                    scalar1=iotac_s[:], scalar2=None,
                                    op0=mybir.AluOpType.is_equal)

                                zc = zc_p.tile([128, 2, 256], f32, tag="zc")
                                for k in range(npair):
                                    t = t0 + k
                                    if t < TL:
                                        xj_sl = lo_g[:, 0, (bi * TL + t) * 128:
                                                     (bi * TL + t + 1) * 128]
                                    else:
                                        th = t - TL
                                        xj_sl = hi_g[:, 0, (bi * TH + th) * 128:
                                                     (bi * TH + th + 1) * 128]
                                    ea_sl = ea_t[:, t * 128:(t + 1) * 128]
                                    nc.tensor.matmul(zc[:, k, :], selt[:, k, :],
                                                     y_s[:],
                                                     start=True, stop=False)
                                    nc.tensor.matmul(zc[:, k, :], xj_sl,
                                                     wxj_s[:, layer, :],
                                                     start=False, stop=False)
                                    nc.tensor.matmul(zc[:, k, :], ea_sl,
                                                     wea_s[:, layer, :],
                                                     start=False, stop=True)

                                # zc holds [-z1 | z2] (z1-half weights
                                # sign-flipped on host).
                                # lnb = ln(1 + e^zc) = [softplus(-z1)|softplus(z2)]
                                # msg = softplus(z2) * sigmoid(z1)
                                #     = lnb_hi * exp(-lnb_lo)
                                ez = small_p.tile([128, 2, 256], f32, tag="ez")
                                nc.scalar.activation(ez[:, :npair, :],
                                                     zc[:, :npair, :], AF.Exp)
                                lnb = small_p.tile([128, 2, 256], f32, tag="lnb")
                                nc.scalar.activation(lnb[:, :npair, :],
                                                     ez[:, :npair, :], AF.Ln,
                                                     bias=ones_s[:])
                                sig = small_p.tile([128, 2, 128], f32, tag="sig")
                                nc.scalar.activation(sig[:, :npair, :],
                                                     lnb[:, :npair, 0:128],
                                                     AF.Exp, scale=-1.0)
                                msg = small_p.tile([128, 2, 128], bf, tag="msg")
                                nc.vector.tensor_tensor(
                                    out=msg[:, :npair, :],
                                    in0=sig[:, :npair, :],
                                    in1=lnb[:, :npair, 128:256],
                                    op=mybir.AluOpType.mult)

                                for k in range(npair):
                                    t = t0 + k
                                    nc.tensor.matmul(
                                        agg[:], sel_blk[:, t, :], msg[:, k, :],
                                        start=(t == 0), stop=(t == TPB - 1))

                            # ---- block epilogue: LN + residual + softplus --
                            xold = xio_p.tile([128, 128], f32, tag="xold")
                            eng2 = (nc.scalar, nc.sync)[blk % 2]
                            eng2.dma_start(
                                xold[:], xold_src[blk * 128:(blk + 1) * 128, :])

                            # row sums of agg and agg^2 via ACT accumulators
                            a_s = xio_p.tile([128, 128], f32, tag="as")
                            s1 = stats_p.tile([128, 1], f32, tag="s1")
                            nc.scalar.activation(a_s[:], agg[:], AF.Copy,
                                                 accum_out=s1[:])
                            sq_t = xio_p.tile([128, 128], f32, tag="sqt")
                            s2 = stats_p.tile([128, 1], f32, tag="s2")
                            nc.scalar.activation(sq_t[:], agg[:], AF.Square,
                                                 accum_out=s2[:])
                            mean = stats_p.tile([128, 1], f32, tag="mean")
                            nc.vector.tensor_scalar(
                                out=mean[:], in0=s1[:], scalar1=1.0 / 128.0,
                                scalar2=None, op0=mybir.AluOpType.mult)
                            var = stats_p.tile([128, 1], f32, tag="var")
                            # var = s2/128 - mean^2
                            msq = stats_p.tile([128, 1], f32, tag="msq")
                            nc.vector.tensor_mul(msq[:], mean[:], mean[:])
                            nc.vector.tensor_scalar(
                                out=var[:], in0=s2[:], scalar1=1.0 / 128.0,
                                scalar2=msq[:], op0=mybir.AluOpType.mult,
                                op1=mybir.AluOpType.subtract)
                            # rstd = exp(-0.5 * ln(var + eps))
                            lnv = stats_p.tile([128, 1], f32, tag="lnv")
                            nc.scalar.activation(lnv[:], var[:], AF.Ln,
                                                 bias=eps_s[:])
                            rstd = stats_p.tile([128, 1], f32, tag="rstd")
                            nc.scalar.activation(rstd[:], lnv[:], AF.Exp,
                                                 scale=-0.5)
                            nmr = stats_p.tile([128, 1], f32, tag="nmr")
                            nc.vector.tensor_scalar(
                                out=nmr[:], in0=mean[:], scalar1=-1.0,
                                scalar2=rstd[:], op0=mybir.AluOpType.mult,
                                op1=mybir.AluOpType.mult)

                            # xn = (agg - mean) * rstd  (ACT affine)
                            xn = xio_p.tile([128, 128], f32, tag="xn")
                            nc.scalar.activation(xn[:], a_s[:], AF.Identity,
                                                 scale=rstd[:], bias=nmr[:])
                            nc.vector.tensor_mul(xn[:], xn[:], g_s[:, layer, :])
                            nc.vector.tensor_add(xn[:], xn[:], b_s[:, layer, :])
                            nc.vector.tensor_add(xn[:], xn[:], xold[:])

                            # softplus(xn) = ln(1 + e^{xn})
                            exn = xio_p.tile([128, 128], f32, tag="exn")
                            nc.scalar.activation(exn[:], xn[:], AF.Exp)
                            xnew = xio_p.tile([128, 128], f32, tag="xnew")
                            nc.scalar.activation(xnew[:], exn[:], AF.Ln,
                                                 bias=ones_s[:])
                            if not last:
                                xbf = xio_p.tile([128, 128], bf, tag="xbf")
                                nc.scalar.activation(xbf[:], xnew[:], AF.Copy)
                                eng3 = (nc.sync, nc.scalar)[(blk + 1) % 2]
                                eng3.dma_start(
                                    xown[layer + 1][blk * 128:(blk + 1) * 128, :],
                                    xbf[:])
                                eng4 = (nc.scalar, nc.sync)[(blk + 1) % 2]
                                eng4.dma_start(
                                    xmast[layer][blk * 128:(blk + 1) * 128, :],
                                    xnew[:])
                                # feature-major copy for next layer's Y
                                xt_ps = selt_p.tile([128, 128], f32, tag="st")
                                nc.tensor.transpose(xt_ps[:], xnew[:], ident_s[:])
                                nc.vector.tensor_copy(
                                    fm_next[:, 0, blk * 128:(blk + 1) * 128],
                                    xt_ps[:])
                            else:
                                # pooled per-graph sums: gsum[g,:] += x[slot,:]
                                gsel = xio_p.tile([128, 256], f32, tag="gsel")
                                nc.vector.tensor_scalar(
                                    out=gsel[:], in0=io256_s[:],
                                    scalar1=gid_s[:, blk:blk + 1],
                                    scalar2=None, op0=mybir.AluOpType.is_equal)
                                nc.tensor.matmul(gsum0, gsel[:, 0:128],
                                                 xnew[:], start=(blk == 0),
                                                 stop=(blk == nblk - 1))
                                nc.tensor.matmul(gsum1, gsel[:, 128:256],
                                                 xnew[:], start=(blk == 0),
                                                 stop=(blk == nblk - 1))

                    # ---- exchange (layers 0,1): AllGather next x table ----
                    if not last:
                        nc.gpsimd.collective_compute(
                            "AllGather", mybir.AluOpType.bypass,
                            replica_groups=rg,
                            ins=[xown[layer + 1][:]], outs=[xall[layer + 1][:]])
                        xall3 = xall[layer + 1].rearrange(
                            "(r p) c -> p r c", p=128)
                        HR = ranks // 2
                        nc.sync.dma_start(tab_s[:, :HR, :], xall3[:, :HR, :])
                        nc.scalar.dma_start(tab_s[:, HR:, :], xall3[:, HR:, :])

                # ---- write pooled output ---------------------------------
                gsb = xio_p.tile([128, 256], f32, tag="gsb")
                nc.scalar.activation(gsb[:, 0:128], gsum0, AF.Copy)
                nc.scalar.activation(gsb[:, 128:256], gsum1, AF.Copy)
                nc.sync.dma_start(gfull_d[0:128, :], gsb[:, 0:128])
                nc.sync.dma_start(gfull_d[128:256, :], gsb[:, 128:256])
                nc.gpsimd.collective_compute(
                    "ReduceScatter", mybir.AluOpType.add, replica_groups=rg,
                    ins=[gfull_d], outs=[gpart_d])
                nc.sync.dma_start(gsum_d, gpart_d)
                if dbg:
                    nc.sync.dma_start(dx1_d, xown[1][:])
                    nc.sync.dma_start(dx2_d, xown[2][:])

    nc.compile()
    return nc


# --------------------------------------------------------------------------
# Host preprocessing
# --------------------------------------------------------------------------

def _softplus(x):
    return np.log1p(np.exp(-np.abs(x))) + np.maximum(x, 0.0)


def _snake_slots(n, n_bins):
    """Slot offsets (bin*128 + round) for n items dealt snake-wise, in the
    order of the sorted item list."""
    idx = np.arange(n)
    r = idx // n_bins
    k = idx % n_bins
    bins = np.where(r % 2 == 0, k, n_bins - 1 - k)
    return bins * 128 + r


def _wrap16(arr):
    # [n] int16 -> [16, n/16], idx i at (i%16, i//16)
    return np.ascontiguousarray(arr.reshape(-1, 16).T)


def preprocess(z, R, edge_index, batch, embedding, emb_w, emb_b, conv_w, conv_b,
               ln_g, ln_b, n_nodes=N_NODES, n_cores=N_CORES, nblk=NBLK,
               lo_slots=LO_SLOTS, edge_d=EDGE_D, cutoff=CUTOFF):
    blocks = n_cores * nblk
    slots = blocks * 128
    core_slots = nblk * 128
    lo_blocks = lo_slots // 128
    n_edges = edge_index.shape[1]
    src = np.asarray(edge_index[0], np.int64)
    dst = np.asarray(edge_index[1], np.int64)

    # edge distances on host (smearing runs on device)
    Rf = np.asarray(R, np.float32)
    d = np.linalg.norm(Rf[src] - Rf[dst], axis=-1)  # [E] f32

    # node permutation: balance per-block in-degrees; L = orig nodes < lo_slots
    islo_e = src < lo_slots
    a = np.bincount(dst[islo_e], minlength=n_nodes)
    b = np.bincount(dst[~islo_e], minlength=n_nodes)
    w = a + b
    ordL = np.argsort(-w[:lo_slots], kind="stable")
    ordH = np.argsort(-w[lo_slots:], kind="stable") + lo_slots
    perm = np.full(n_nodes, -1, np.int64)
    perm[ordL] = _snake_slots(ordL.size, lo_blocks)
    perm[ordH] = _snake_slots(ordH.size, blocks - lo_blocks) + lo_slots
    assert perm.min() >= 0

    es, ed = perm[src], perm[dst]
    blk = ed // 128

    lo_cnt = np.bincount(blk[islo_e], minlength=blocks)
    hi_cnt = np.bincount(blk[~islo_e], minlength=blocks)
    TL = int(-(-lo_cnt.max() // 128))
    TH = int(-(-hi_cnt.max() // 128))
    TPB = TL + TH
    S = nblk * TPB * 128

    # edge slot assignment: within block, lows first then highs
    key = blk * 2 + (~islo_e).astype(np.int64)
    eorder = np.argsort(key, kind="stable")
    ks = key[eorder]
    runstart = np.r_[0, np.flatnonzero(np.diff(ks)) + 1]
    runid = np.zeros(n_edges, np.int64)
    runid[runstart[1:]] = 1
    runid = np.cumsum(runid)
    pos = np.arange(n_edges) - runstart[runid]
    eb = ks // 2
    ehalf = ks % 2
    base = eb * TPB * 128 + ehalf * (TL * 128)
    eslot_g = base + pos
    core_of = eb // nblk
    eslot = eslot_g - core_of * (nblk * TPB * 128)

    ixlo = np.zeros((n_cores, nblk * TL * 128), np.int16)
    ixhi = np.zeros((n_cores, nblk * TH * 128), np.int16)
    dstv = np.full((n_cores, nblk * TPB, 128), -1.0, np.float32)
    u = np.zeros((n_cores, 1, S), np.float32)

    e_src = es[eorder]
    e_dst = ed[eorder]
    e_lo = ehalf == 0
    d_o = d[eorder]

    for c in range(n_cores):
        m = core_of == c
        sl = eslot[m]
        # xj
        mlo = m & e_lo
        mhi = m & ~e_lo
        slo_ = eslot[mlo]
        bb = slo_ // (TPB * 128)
        off = slo_ - bb * (TPB * 128)
        ixlo[c][bb * TL * 128 + off] = e_src[mlo].astype(np.int16)
        shi_ = eslot[mhi]
        bb = shi_ // (TPB * 128)
        off = shi_ - bb * (TPB * 128) - TL * 128
        ixhi[c][bb * TH * 128 + off] = (e_src[mhi] - lo_slots).astype(np.int16)
        # dst one-hot value, edge distances
        dstv[c].reshape(-1)[sl] = (e_dst[m] % 128).astype(np.float32)
        u[c, 0, sl] = d_o[m].astype(np.float32)

    # z tables (slot -> atom type; empty slots -> 100 which maps to a 0 row)
    zslot = np.full(slots, 100, np.int16)
    zslot[perm] = np.asarray(z, np.int16)
    # graph-id per slot (empty -> -1, excluded from pooling)
    gslot = np.full(slots, -1.0, np.float32)
    gslot[perm] = np.asarray(batch, np.float32)

    # embedding rows
    EWf = np.zeros((128, 128), np.float32)
    EWf[:100] = (np.asarray(embedding, np.float32)
                 @ np.asarray(emb_w, np.float32)
                 + np.asarray(emb_b, np.float32))
    EWb = EWf.astype(BF16)

    # conv weights; z1-half output columns sign-flipped so the device computes
    # [-z1 | z2] and can use exp/ln-only activations (one act table)
    cw = np.asarray(conv_w, np.float32).copy()
    cb = np.asarray(conv_b, np.float32).copy()
    cw[:, :, :128] *= -1.0
    cb[:, :128] *= -1.0
    wxi = np.ascontiguousarray(cw[:, :128, :].transpose(1, 0, 2)).astype(BF16)
    wxj = np.ascontiguousarray(cw[:, 128:256, :].transpose(1, 0, 2)).astype(BF16)
    wea = np.concatenate([cw[:, 256:, :], cb[:, None, :]], axis=1)
    wea = np.ascontiguousarray(wea.transpose(1, 0, 2)).astype(BF16)

    # LN gamma/beta rows
    lnr = np.concatenate(
        [np.concatenate([np.asarray(ln_g, np.float32)[l],
                         np.asarray(ln_b, np.float32)[l]])
         for l in range(cw.shape[0])])[None, :]

    # smearing: ea_k = exp(cfs_k * (d - offs_k)^2); cfs[100]=0 -> bias row 1
    offs = np.linspace(0.0, cutoff, edge_d, dtype=np.float32)
    coeff = np.float32(-0.5 / (offs[1] - offs[0]) ** 2)
    noffs = np.zeros((101, 1), np.float32)
    noffs[:edge_d, 0] = -offs
    cfs = np.zeros((101, 1), np.float32)
    cfs[:edge_d, 0] = coeff

    ior = np.arange(256, dtype=np.float32)[None, :]

    # ---- pack blobs ------------------------------------------------------
    uents, sents, BU, BS, BS8 = _layout(TL, TH, nblk, ranks=blocks,
                                        n_cores=n_cores)

    def pack(ents, arrays, nbytes):
        blob = np.zeros(nbytes // 2, np.int16)
        bv = blob.view(np.uint8)
        for name, (off, shape, dt_) in ents.items():
            a = np.ascontiguousarray(arrays[name])
            assert a.shape == tuple(shape) and a.dtype == np.dtype(dt_), \
                (name, a.shape, shape, a.dtype, dt_)
            bv[off:off + a.nbytes] = a.view(np.uint8).ravel()
        return blob

    sblob = pack(sents, {
        "ewb": EWb, "ewf": EWf,
        "wxi": wxi, "wxj": wxj, "wea": wea,
        "lnr": lnr, "ior": ior, "noffs": noffs, "cfs": cfs,
    }, BS)

    in_maps = []
    for c in range(n_cores):
        sl0 = c * core_slots
        uq = np.round(np.minimum(u[c], UQMAX) * (32767.0 / UQMAX)
                      ).astype(np.int16)
        ublob = pack(uents, {
            "u": uq,
            "ixlo": _wrap16(ixlo[c]),
            "ixhi": _wrap16(ixhi[c]),
            "zown": _wrap16(zslot[sl0:sl0 + core_slots]),
            "dstv": np.ascontiguousarray(
                dstv[c].transpose(1, 0)).astype(np.int8),
            "gid": np.ascontiguousarray(
                gslot[sl0:sl0 + core_slots].reshape(nblk, 128).T),
        }, BU)
        in_maps.append({
            "ublob": ublob,
            "sblob": sblob[c * BS8 // 2:(c + 1) * BS8 // 2],
        })
    return in_maps, TL, TH


# --------------------------------------------------------------------------
# execution: cached jitted SPMD runner (PJRT via bass2jax custom call)
# --------------------------------------------------------------------------

class _Results:
    """Minimal stand-in for BassKernelResults (test.py reads exec_time_ns)."""

    def __init__(self, results):
        self.results = results
        self.exec_time_ns = None


class _Runner:
    """Compile once, then run full numpy in_maps -> numpy outputs."""

    def __init__(self, nc, n_cores):
        import jax
        from jax.sharding import Mesh, PartitionSpec
        from jax.experimental.shard_map import shard_map
        from concourse import bass2jax

        bass2jax.install_neuronx_cc_hook()
        self.nc = nc
        self.n_cores = n_cores
        partition_name = (nc.partition_id_tensor.name
                          if nc.partition_id_tensor else None)
        in_names, out_names, out_avals, zero_outs = [], [], [], []
        for alloc in nc.m.functions[0].allocations:
            if not isinstance(alloc, mybir.MemoryLocationSet):
                continue
            name = alloc.memorylocations[0].name
            if alloc.kind == "ExternalInput":
                if name != partition_name:
                    in_names.append(name)
            elif alloc.kind == "ExternalOutput":
                shape = tuple(alloc.tensor_shape)
                dtype = mybir.dt.np(alloc.dtype)
                out_names.append(name)
                out_avals.append(jax.core.ShapedArray(shape, dtype))
                zero_outs.append(np.zeros((n_cores * shape[0], *shape[1:]),
                                          dtype))
        self.in_names = in_names
        self.out_names = out_names
        self.out_shapes = [tuple(a.shape) for a in out_avals]
        self.zero_outs = zero_outs
        n_params = len(in_names)
        all_in = in_names + out_names + (
            [partition_name] if partition_name else [])

        def _body(*args):
            operands = list(args)
            if partition_name is not None:
                operands.append(bass2jax.partition_id_tensor())
            outs = bass2jax._bass_exec_p.bind(
                *operands, out_avals=tuple(out_avals),
                in_names=tuple(all_in), out_names=tuple(out_names),
                lowering_input_output_aliases=(),
                sim_require_finite=True, sim_require_nnan=True, nc=nc)
            return tuple(outs)

        devs = jax.devices()[:n_cores]
        assert len(devs) == n_cores
        mesh = Mesh(np.asarray(devs), ("core",))
        n_outs = len(out_avals)
        self._fn = jax.jit(
            shard_map(_body, mesh=mesh,
                      in_specs=(PartitionSpec("core"),) * (n_params + n_outs),
                      out_specs=(PartitionSpec("core"),) * n_outs,
                      check_rep=False),
            donate_argnums=tuple(range(n_params, n_params + n_outs)),
            keep_unused=True)
    def run(self, in_maps):
        concat_in = [
            np.concatenate([np.asarray(m[n]) for m in in_maps], axis=0)
            for n in self.in_names]
        outs = self._fn(*concat_in, *self.zero_outs)
        n = self.n_cores
        return _Results([
            {name: np.asarray(outs[i]).reshape(n, *self.out_shapes[i])[c]
             for i, name in enumerate(self.out_names)}
            for c in range(n)])


def kernel(z, R, edge_index, batch, embedding, emb_w, emb_b, conv_w, conv_b,
           ln_g, ln_b, cfc_w, cfc_b, fc_w, fc_b, out_w, out_b):
    in_maps, TL, TH = preprocess(
        z, R, edge_index, batch, embedding, emb_w, emb_b, conv_w, conv_b,
        ln_g, ln_b)

    key = (TL, TH)
    if key not in _NC_CACHE:
        nc = build_nc(TL, TH)
        _NC_CACHE[key] = _Runner(nc, N_CORES)
    runner = _NC_CACHE[key]

    res = runner.run(in_maps)
    global LAST_RESULTS, LAST_RERUN_S, LAST_RUN
    LAST_RUN = (runner, in_maps)
    LAST_RESULTS = res
    if _os.environ.get("KERNEL_RERUN", "1") != "0":
        import time as _time
        t0 = _time.time()
        runner.run(in_maps)
        LAST_RERUN_S = _time.time() - t0

    gs = np.concatenate([res.results[c]["gsum"] for c in range(N_CORES)],
                        axis=0)  # [256, 128] fully-summed (reduce-scattered)

    batch = np.asarray(batch, np.int64)
    cnts = np.bincount(batch, minlength=N_GRAPHS).astype(np.float32)
    mol = gs / np.maximum(cnts, 1.0)[:, None]

    h = _softplus(mol @ np.asarray(cfc_w, np.float32) + np.asarray(cfc_b, np.float32))
    for l in range(np.asarray(fc_w).shape[0]):
        h = _softplus(h @ np.asarray(fc_w[l], np.float32)
                      + np.asarray(fc_b[l], np.float32))
    out = h @ np.asarray(out_w, np.float32) + np.asarray(out_b, np.float32)
    return out.astype(np.float32)


# revision 24
# speedup vs baseline: 2.1971x; 1.1820x over previous
"""CGCNN (no BN) message-passing GNN on 8 Trainium2 NeuronCores.

Strategy (self-contained; shapes hardcoded from the problem spec):
 - Nodes are permuted on the host into 392 blocks of 128 slots, balancing
   per-block in-edge counts. Cores own 49 contiguous blocks (6272 slots).
 - Edges are owned by the core that owns their destination block; within a
   block, edges are split by source-slot half (<32768 vs >=32768) so each
   128-edge tile gathers from a single int16-indexable table view, then
   padded to a uniform (TL, TH) tile count per block so all 8 cores run one
   SPMD program.
 - Node-feature tables live in DRAM (slot-major [slots, 128] bf16); per-tile
   x[dst] / x[src] columns are pulled with DRAM-source transposed dma_gather
   (7-block chunks amortize the large fixed cost per gather call).
 - All inner-loop activations use only exp/ln/copy/square so a single ACT
   table set serves the whole kernel (a get_activation_tables patch makes
   the table-load pass pick the combined natural_log_exp set -> no
   ACT_TABLE_LOAD thrash).  msg = softplus(z2)*sigmoid(z1) is computed as
   lnb = ln(1+exp([-z1|z2])), msg = lnb_hi * exp(-lnb_lo) -- no DVE
   reciprocal.
 - LayerNorm epilogue uses ACT accum_out row-sums (mean / mean-of-squares)
   and an ACT affine (scale/bias per partition) instead of bn_aggr and the
   slow dual-scalar tensor_scalar.
 - Per-graph mean-pooling via one-hot matmuls accumulated in PSUM on the
   last layer; each core returns a reduce-scattered [32, 128] partial.
 - The tiny pooled-MLP head runs on the host in f32.
"""

import functools as _functools
import os as _os
import numpy as np
import ml_dtypes

import concourse.hw_specs as _hw_specs
import concourse.bacc as _bacc_mod

# --- activation-table patch: make exp and ln resolve to the same table set
# (natural_log_exp_and_others) so the ACT engine never reloads tables in the
# inner loop.  Only empties the shadowing set entries; ids/order unchanged.
_ORIG_ACT_TABLES = _hw_specs.get_activation_tables


@_functools.cache
def _act_tables_patched(arch):
    t = dict(_ORIG_ACT_TABLES(arch))
    for name in ("exp_and_others", "natural_log"):
        if name in t:
            t[name] = set()
    return t


_hw_specs.get_activation_tables = _act_tables_patched
_bacc_mod.get_activation_tables = _act_tables_patched

import concourse.bass as bass
import concourse.tile as tile
from concourse import bacc, mybir

BF16 = ml_dtypes.bfloat16

# Problem constants
N_NODES, N_EDGES, NODE_D, EDGE_D, EMB_D, N_GRAPHS = 50000, 800000, 128, 100, 92, 256
N_CONV, FC_D, N_FC, CUTOFF = 3, 128, 2, 6.0

LAST_RESULTS = None        # BassKernelResults of the most recent run (for tests)
LAST_RERUN_S = None        # wall seconds of a warm re-execution
LAST_RUN = None            # (runner, in_maps) of the most recent run (for tests)

N_CORES = 8
UQMAX = 6.625              # d quantization range (beyond it all gaussians ~ 0)
SLOTS = 50176              # 392 blocks * 128
BLOCKS = SLOTS // 128      # 392
NBLK = BLOCKS // N_CORES   # 49 blocks per core
CORE_SLOTS = NBLK * 128    # 6272
LO_SLOTS = 32768           # slots gatherable from the low table view
CHUNK = 3                  # blocks processed per gather chunk

_NC_CACHE = {}


# --------------------------------------------------------------------------
# Input blob layout (shared between host packing and device program)
# --------------------------------------------------------------------------

def _layout(TL, TH, nblk=NBLK, ranks=BLOCKS, n_cores=N_CORES):
    """Byte layout of the per-core-unique and shared input blobs."""
    TPB = TL + TH
    NT = nblk * TPB
    S = NT * 128
    SLO = nblk * TL * 128
    SHI = nblk * TH * 128
    core_slots = nblk * 128

    uspec = [
        ("u", (1, S), np.int16),        # d quantized to [0, UQMAX]
        ("ixlo", (16, SLO // 16), np.int16),
        ("ixhi", (16, SHI // 16), np.int16),
        ("zown", (16, core_slots // 16), np.int16),
        ("dstv", (128, NT), np.int8),
        ("gid", (128, nblk), np.float32),
    ]
    sspec = [
        ("ewb", (128, 128), BF16),
        ("ewf", (128, 128), np.float32),
        ("wxi", (128, N_CONV, 256), BF16),
        ("wxj", (128, N_CONV, 256), BF16),
        ("wea", (101, N_CONV, 256), BF16),
        ("lnr", (1, N_CONV * 256), np.float32),
        ("ior", (1, 256), np.float32),
        ("noffs", (101, 1), np.float32),
        ("cfs", (101, 1), np.float32),
    ]

    def place(spec):
        ents, off = {}, 0
        for name, shape, dt_ in spec:
            nbytes = int(np.prod(shape)) * np.dtype(dt_).itemsize
            ents[name] = (off, shape, dt_)
            off += -(-nbytes // 512) * 512
        return ents, off

    uents, BU = place(uspec)
    sents, BS = place(sspec)
    BU = -(-BU // 512) * 512
    align = n_cores * 512
    BS = -(-BS // align) * align
    return uents, sents, BU, BS, BS // n_cores


_NP2MY = None


def _my_dt(np_dt):
    global _NP2MY
    if _NP2MY is None:
        _NP2MY = {np.dtype(np.float32): mybir.dt.float32,
                  np.dtype(np.int16): mybir.dt.int16,
                  np.dtype(np.int8): mybir.dt.int8,
                  np.dtype(np.uint8): mybir.dt.uint8,
                  np.dtype(BF16): mybir.dt.bfloat16}
    return _NP2MY[np.dtype(np_dt)]


# --------------------------------------------------------------------------
# Device program
# --------------------------------------------------------------------------

def build_nc(TL, TH, nblk=NBLK, ranks=BLOCKS, n_cores=N_CORES,
             lo_ranks=LO_SLOTS // 128):
    """Build the SPMD Bass program. TL/TH = low/high tiles per block."""
    TPB = TL + TH                 # tiles per block
    NT = nblk * TPB               # tiles per core
    S = NT * 128                  # edge slots per core
    SLO = nblk * TL * 128
    SHI = nblk * TH * 128
    slots = ranks * 128
    core_slots = nblk * 128
    lo_slots = lo_ranks * 128
    f32, bf, i16 = mybir.dt.float32, mybir.dt.bfloat16, mybir.dt.int16
    AF = mybir.ActivationFunctionType

    nc = bacc.Bacc("TRN2", target_bir_lowering=False, debug=False,
                   num_devices=n_cores)

    # ---- external inputs: two flat blobs ---------------------------------
    uents, sents, BU, BS, BS8 = _layout(TL, TH, nblk, ranks, n_cores)
    ublob_d = nc.dram_tensor("ublob", [BU // 2], i16, kind="ExternalInput").ap()
    sblob_d = nc.dram_tensor("sblob", [BS8 // 2], i16, kind="ExternalInput").ap()
    sstage_d = nc.dram_tensor("sstage", [BS8 // 2], i16, kind="Internal").ap()
    sall_d = nc.dram_tensor("sall", [BS // 2], i16, kind="Internal",
                            addr_space="Shared").ap()

    def V(ents, base):
        def view(name):
            off, shape, dt_ = ents[name]
            nbytes = int(np.prod(shape)) * np.dtype(dt_).itemsize
            v = base[off // 2: (off + nbytes) // 2].bitcast(_my_dt(dt_))
            if len(shape) == 2:
                v = v.rearrange("(a b) -> a b", b=shape[1])
            elif len(shape) == 3:
                v = v.rearrange("(a b c) -> a b c", b=shape[1], c=shape[2])
            return v
        return view

    UV = V(uents, ublob_d)
    SV = V(sents, sall_d)
    u_d = UV("u")
    ixlo_d, ixhi_d = UV("ixlo"), UV("ixhi")
    zown_d, dst_d, gid_d = UV("zown"), UV("dstv"), UV("gid")
    ewb_d, ewf_d = SV("ewb"), SV("ewf")
    wxi_d, wxj_d, wea_d = SV("wxi"), SV("wxj"), SV("wea")
    lnr_d, ior_d = SV("lnr"), SV("ior")
    noffs_d, cfs_d = SV("noffs"), SV("cfs")

    # ---- internal DRAM ---------------------------------------------------
    ea_h = nc.dram_tensor("ea_h", [101, S], bf, kind="Internal").ap()
    ixlo_x = nc.dram_tensor("ixlo_x", [128, SLO // 16], i16, kind="Internal").ap()
    ixhi_x = nc.dram_tensor("ixhi_x", [128, SHI // 16], i16, kind="Internal").ap()
    zown_x = nc.dram_tensor("zown_x", [128, core_slots // 16], i16,
                            kind="Internal").ap()
    xinit = nc.dram_tensor("xinit", [core_slots, 128], f32, kind="Internal").ap()
    xmast = [
        nc.dram_tensor(f"xmast{i}", [core_slots, 128], f32, kind="Internal").ap()
        for i in range(2)
    ]
    # per-core slot-major bf16 x tables: xown[l] = this core's x^(l) slice
    xown = [
        nc.dram_tensor(f"xown{i}", [core_slots, 128], bf, kind="Internal").ap()
        for i in range(3)
    ]
    xall = [
        nc.dram_tensor(f"xall{i}", [n_cores * core_slots, 128], bf,
                       kind="Internal", addr_space="Shared").ap()
        for i in range(3)
    ]
    gfull_d = nc.dram_tensor("gfull", [256, 128], f32, kind="Internal").ap()
    gpart_d = nc.dram_tensor("gpart", [256 // n_cores, 128], f32,
                             kind="Internal").ap()
    gsum_d = nc.dram_tensor("gsum", [256 // n_cores, 128], f32,
                            kind="ExternalOutput").ap()
    dbg = _os.environ.get("KERNEL_DEBUG", "0") == "1"
    if dbg:
        dx1_d = nc.dram_tensor("dx1", [core_slots, 128], bf,
                               kind="ExternalOutput").ap()
        dx2_d = nc.dram_tensor("dx2", [core_slots, 128], bf,
                               kind="ExternalOutput").ap()

    rg = [list(range(n_cores))]

    with tile.TileContext(nc) as tc:
        # reassemble the shared blob from its 8 per-core slices
        nc.sync.dma_start(sstage_d, sblob_d)
        nc.gpsimd.collective_compute(
            "AllGather", mybir.AluOpType.bypass, replica_groups=rg,
            ins=[sstage_d], outs=[sall_d])

        with tc.tile_pool(name="persist", bufs=1) as persist:
            tab_s = persist.tile([128, ranks, 128], bf)
            loc_fm = [persist.tile([128, 1, nblk * 128], bf, tag=f"fm{i}",
                                   name=f"loc_fm{i}")
                      for i in range(2)]
            ident_s = persist.tile([128, 128], f32)
            iotac_s = persist.tile([128, 1], f32)
            dst_s = persist.tile([128, NT], f32)
            iota_s = persist.tile([128, 128], bf)
            io256_s = persist.tile([128, 256], bf)
            wxi_s = persist.tile([128, N_CONV, 256], bf)
            wxj_s = persist.tile([128, N_CONV, 256], bf)
            wea_s = persist.tile([101, N_CONV, 256], bf)
            g_s = persist.tile([128, N_CONV, 128], f32)
            b_s = persist.tile([128, N_CONV, 128], f32)
            gid_s = persist.tile([128, nblk], f32)
            eps_s = persist.tile([128, 1], f32)
            ones_s = persist.tile([128, 1], f32)

            nc.sync.dma_start(wxi_s[:], wxi_d)
            nc.sync.dma_start(wxj_s[:], wxj_d)
            nc.sync.dma_start(wea_s[:], wea_d)
            nc.sync.dma_start(gid_s[:], gid_d)
            nc.vector.memset(eps_s[:], 1e-5)
            nc.vector.memset(ones_s[:], 1.0)

            # ---------------- init: expansions + tables + edge features --
            with (
                tc.tile_pool(name="initp", bufs=1) as initp,
                tc.tile_pool(name="inits", bufs=3) as inits,
                tc.tile_pool(name="initps", bufs=2, space="PSUM") as initps,
            ):
                noffs_s = initp.tile([101, 1], f32, tag="noffs")
                cfs_s = initp.tile([101, 1], f32, tag="cfs")
                lnr_s = initp.tile([1, N_CONV * 256], f32, tag="lnr")
                ior_s = initp.tile([1, 256], f32, tag="ior")
                one1_s = initp.tile([1, 128], f32, tag="one1")
                on101_s = initp.tile([1, 101], f32, tag="on101")
                dstb_s = initp.tile([128, NT], mybir.dt.int8, tag="dstb")
                nc.sync.dma_start(noffs_s[:], noffs_d)
                nc.sync.dma_start(cfs_s[:], cfs_d)
                nc.sync.dma_start(lnr_s[:], lnr_d)
                nc.sync.dma_start(ior_s[:], ior_d)
                nc.vector.memset(one1_s[:], 1.0)
                nc.vector.memset(on101_s[:], 1.0)
                nc.sync.dma_start(dstb_s[:], dst_d)
                nc.vector.tensor_scalar(
                    out=dst_s[:], in0=dstb_s[:], scalar1=1.0, scalar2=None,
                    op0=mybir.AluOpType.mult)

                # replicate [16, W] index arrays into [128, W] DRAM via SBUF
                STW = 1024
                for src_d, dst_x, w in (
                    (ixlo_d, ixlo_x, SLO // 16),
                    (ixhi_d, ixhi_x, SHI // 16),
                    (zown_d, zown_x, core_slots // 16),
                ):
                    for o in range(0, w, STW):
                        ww = min(STW, w - o)
                        st = inits.tile([16, STW], i16, tag="ix_st")
                        nc.sync.dma_start(st[:, :ww], src_d[:, o:o + ww])
                        for k in range(8):
                            eng = (nc.sync, nc.scalar)[k % 2]
                            eng.dma_start(
                                dst_x[k * 16:(k + 1) * 16, o:o + ww],
                                st[:, :ww])

                # iota / iota256 built by broadcasting a row over partitions
                pio = initps.tile([128, 512], f32, tag="pio")
                nc.tensor.matmul(pio[:, :256], one1_s[:], ior_s[:],
                                 start=True, stop=True)
                nc.scalar.activation(io256_s[:], pio[:, :256], AF.Copy)
                nc.scalar.activation(iota_s[:], pio[:, :128], AF.Copy)
                iotac_i = initp.tile([128, 1], mybir.dt.int16, tag="ioci")
                nc.gpsimd.iota(iotac_i[:], [[1, 1]], base=0,
                               channel_multiplier=1)
                nc.vector.tensor_scalar(
                    out=iotac_s[:], in0=iotac_i[:], scalar1=1.0, scalar2=None,
                    op0=mybir.AluOpType.mult)
                iotaf_s = initp.tile([128, 128], f32, tag="iotaf")
                nc.scalar.activation(iotaf_s[:], pio[:, :128], AF.Copy)
                nc.vector.tensor_scalar(
                    out=ident_s[:], in0=iotaf_s[:], scalar1=iotac_s[:],
                    scalar2=None, op0=mybir.AluOpType.is_equal)
                # LN gamma/beta broadcast
                for l in range(N_CONV):
                    pln = initps.tile([128, 512], f32, tag="pln")
                    nc.tensor.matmul(pln[:, :256], one1_s[:],
                                     lnr_s[:, l * 256:(l + 1) * 256],
                                     start=True, stop=True)
                    nc.scalar.activation(g_s[:, l, :], pln[:, :128], AF.Copy)
                    nc.scalar.activation(b_s[:, l, :], pln[:, 128:256], AF.Copy)

                # z-indexed gathers build this core's x0 slice from the
                # embedding; slot-major staging -> DRAM; AllGather -> xall0
                zownt = initp.tile([128, core_slots // 16], i16, tag="zown")
                nc.sync.dma_start(zownt[:], zown_x)
                PIECE = 4096   # HW limit: big single gathers crash the device
                xb_t = initp.tile([128, nblk, 128], bf, tag="xb")
                x0_t = initp.tile([128, nblk, 128], f32, tag="x0")
                fm0 = loc_fm[0]
                # critical path first: own bf16 slice -> DRAM -> AllGather
                for o in range(0, core_slots, PIECE):
                    n = min(PIECE, core_slots - o)
                    nc.gpsimd.dma_gather(
                        xb_t[:, o // 128:(o + n) // 128, :], ewb_d,
                        zownt[:, o // 16:(o + n) // 16], n, n, 128,
                        transpose=False, single_packet=False)
                nc.sync.dma_start(xown[0].rearrange("(r p) c -> p r c", p=128),
                                  xb_t[:])
                nc.gpsimd.collective_compute(
                    "AllGather", mybir.AluOpType.bypass, replica_groups=rg,
                    ins=[xown[0][:]], outs=[xall[0][:]])
                # off the critical path: f32 residual + feature-major copies
                for o in range(0, core_slots, PIECE):
                    n = min(PIECE, core_slots - o)
                    nc.gpsimd.dma_gather(
                        x0_t[:, o // 128:(o + n) // 128, :], ewf_d,
                        zownt[:, o // 16:(o + n) // 16], n, n, 128,
                        transpose=False, single_packet=False)
                    nc.gpsimd.dma_gather(
                        fm0[:, :, o:o + n], ewb_d,
                        zownt[:, o // 16:(o + n) // 16], n, n, 128,
                        transpose=True, single_packet=False)
                nc.scalar.dma_start(xinit.rearrange("(r p) c -> p r c", p=128),
                                  x0_t[:])
                xall3 = xall[0].rearrange("(r p) c -> p r c", p=128)
                TR = (ranks - 120) // 2
                nc.sync.dma_start(tab_s[:, :TR, :], xall3[:, :TR, :])
                nc.scalar.dma_start(tab_s[:, TR:2 * TR, :], xall3[:, TR:2 * TR, :])
                nc.gpsimd.dma_start(tab_s[:, 2 * TR:, :], xall3[:, 2 * TR:, :])

                # gaussian smearing: ea_k = exp(cfs_k * (d - offs_k)^2);
                # row 100 has cfs=0 -> exp(0)=1 (the conv-bias row).
                P = 512
                for off in range(0, S, P):
                    w = min(P, S - off)
                    uti = inits.tile([1, P], i16, tag="uti")
                    seng = (nc.sync, nc.scalar)[(off // P) % 2]
                    seng.dma_start(uti[:, :w], u_d[:, off:off + w])
                    ut = inits.tile([1, P], f32, tag="ut")
                    nc.vector.tensor_scalar(
                        out=ut[:, :w], in0=uti[:, :w],
                        scalar1=float(UQMAX / 32767.0), scalar2=None,
                        op0=mybir.AluOpType.mult)
                    pe_ = initps.tile([101, P], f32, tag="pea")
                    nc.tensor.matmul(pe_[:, :w], on101_s[:], ut[:, :w],
                                     start=True, stop=True)
                    sq = inits.tile([101, P], f32, tag="sq")
                    nc.scalar.activation(sq[:, :w], pe_[:, :w], AF.Square,
                                         bias=noffs_s[:])
                    et = inits.tile([101, P], bf, tag="et")
                    nc.scalar.activation(et[:, :w], sq[:, :w], AF.Exp,
                                         scale=cfs_s[:])
                    seng2 = (nc.scalar, nc.sync)[(off // P) % 2]
                    seng2.dma_start(ea_h[:, off:off + w], et[:, :w])

            # ---------------- main conv layers ----------------------------
            with (
                tc.tile_pool(name="glo", bufs=2) as glo_p,
                tc.tile_pool(name="ghi", bufs=2) as ghi_p,
                tc.tile_pool(name="eat", bufs=2) as ea_p,
                tc.tile_pool(name="idx", bufs=2) as idx_p,
                tc.tile_pool(name="small", bufs=2) as small_p,
                tc.tile_pool(name="selb", bufs=2) as selb_p,
                tc.tile_pool(name="xio", bufs=2) as xio_p,
                tc.tile_pool(name="stats", bufs=2) as stats_p,
                tc.tile_pool(name="zc", bufs=2, space="PSUM") as zc_p,
                tc.tile_pool(name="agg", bufs=2, space="PSUM") as agg_p,
                tc.tile_pool(name="yp", bufs=1, space="PSUM") as y_p,
                tc.tile_pool(name="selt", bufs=1, space="PSUM") as selt_p,
                tc.tile_pool(name="gsm", bufs=1, space="PSUM") as gsm_p,
            ):
                n_chunks = (nblk + CHUNK - 1) // CHUNK
                tab_flat = tab_s.rearrange("p r c -> p (r c)")
                tab_lo_view = tab_flat[:, : lo_ranks * 128]
                tab_hi_view = tab_flat[:, lo_ranks * 128:]
                gsum0_t = gsm_p.tile([128, 128], f32, tag="g0")
                gsum1_t = gsm_p.tile([128, 128], f32, tag="g1")
                gsum0 = gsum0_t[:]
                gsum1 = gsum1_t[:]

                for layer in range(N_CONV):
                    last = layer == N_CONV - 1
                    xold_src = xinit if layer == 0 else xmast[layer - 1]
                    fm_cur = loc_fm[layer % 2]
                    fm_next = loc_fm[(layer + 1) % 2]

                    for ch in range(n_chunks):
                        b0 = ch * CHUNK
                        nb = min(CHUNK, nblk - b0)
                        n_tl = nb * TL
                        n_th = nb * TH

                        # ---- per-chunk index loads -------------------------
                        ixlo_t = idx_p.tile([128, CHUNK * TL * 8], i16, tag="ixlo")
                        ixhi_t = idx_p.tile([128, CHUNK * TH * 8], i16, tag="ixhi")
                        nc.sync.dma_start(
                            ixlo_t[:, :n_tl * 8],
                            ixlo_x[:, b0 * TL * 8: b0 * TL * 8 + n_tl * 8])
                        nc.sync.dma_start(
                            ixhi_t[:, :n_th * 8],
                            ixhi_x[:, b0 * TH * 8: b0 * TH * 8 + n_th * 8])

                        # ---- xj gathers (SBUF-source, transposed, bf16) ----
                        lo_g = glo_p.tile([128, 1, CHUNK * TL * 128], bf, tag="lo")
                        hi_g = ghi_p.tile([128, 1, CHUNK * TH * 128], bf, tag="hi")
                        nc.gpsimd.dma_gather(
                            lo_g[:, :, :n_tl * 128], tab_lo_view,
                            ixlo_t[:, :n_tl * 8],
                            n_tl * 128, n_tl * 128, 128,
                            transpose=True, sbuf_tokens_per_rank=128,
                            sbuf_free_dim_per_rank=256, single_packet=False)
                        nc.gpsimd.dma_gather(
                            hi_g[:, :, :n_th * 128], tab_hi_view,
                            ixhi_t[:, :n_th * 8],
                            n_th * 128, n_th * 128, 128,
                            transpose=True, sbuf_tokens_per_rank=128,
                            sbuf_free_dim_per_rank=256, single_packet=False)

                        # ---- per-block compute -----------------------------
                        for bi in range(nb):
                            blk = b0 + bi
                            # edge features for this block (small DMA,
                            # alternate issuing engine to spread hw queues)
                            ea_t = ea_p.tile([101, TPB * 128], bf, tag="ea")
                            eng = (nc.sync, nc.scalar)[blk % 2]
                            eng.dma_start(
                                ea_t[:],
                                ea_h[:, blk * TPB * 128:(blk + 1) * TPB * 128])

                            # Y = x_block^T @ W1  (replaces the x[dst] gather:
                            # zc1 = selT @ Y gathers rows of Y by dst slot)
                            y_ps = y_p.tile([128, 256], f32, tag="y")
                            nc.tensor.matmul(
                                y_ps[:], fm_cur[:, 0, blk * 128:(blk + 1) * 128],
                                wxi_s[:, layer, :], start=True, stop=True)
                            y_s = xio_p.tile([128, 256], bf, tag="ys")
                            nc.scalar.activation(y_s[:], y_ps[:], AF.Copy)

                            # one-hot dst matrices for all tiles of the
                            # block in a single batched is_equal
                            sel_blk = selb_p.tile([128, TPB, 128], bf, tag="sb")
                            dsl = dst_s[:, blk * TPB:(blk + 1) * TPB]
                            nc.vector.tensor_tensor(
                                out=sel_blk[:],
                                in0=iota_s[:, None, :].broadcast_to(
                                    [128, TPB, 128]),
                                in1=dsl[:, :, None].broadcast_to(
                                    [128, TPB, 128]),
                                op=mybir.AluOpType.is_equal)

                            agg = agg_p.tile([128, 128], f32, tag="agg")
                            pairs = [(t, min(t + 1, TPB - 1)) if t + 1 < TPB
                                     else (t, t) for t in range(0, TPB, 2)]
                            for t0 in range(0, TPB, 2):
                                npair = min(2, TPB - t0)
                                # transposed one-hots (two closed PE groups
                                # into slices of one PSUM bank)
                                st_ps = selt_p.tile([128, 2, 128], f32, tag="st")
                                for k in range(npair):
                                    dcol = dst_s[:, blk * TPB + t0 + k:
                                                 blk * TPB + t0 + k + 1]
                                    nc.tensor.transpose(
                                        st_ps[:, k, :],
                                        dcol.to_broadcast([128, 128]),
                                        ident_s[:])
                                selt = small_p.tile([128, 2, 128], bf, tag="selt")
                                nc.vector.tensor_scalar(
                                    out=selt[:, :npair, :],
                                    in0=st_ps[:, :npair, :],
                                    scalar1=iotac_s[:], scalar2=None,
                                    op0=mybir.AluOpType.is_equal)

                                zc = zc_p.tile([128, 2, 256], f32, tag="zc")
                                for k in range(npair):
                                    t = t0 + k
                                    if t < TL:
                                        xj_sl = lo_g[:, 0, (bi * TL + t) * 128:
                                                     (bi * TL + t + 1) * 128]
                                    else:
                                        th = t - TL
                                        xj_sl = hi_g[:, 0, (bi * TH + th) * 128:
                                                     (bi * TH + th + 1) * 128]
                                    ea_sl = ea_t[:, t * 128:(t + 1) * 128]
                                    nc.tensor.matmul(zc[:, k, :], selt[:, k, :],
                                                     y_s[:],
                                                     start=True, stop=False)
                                    nc.tensor.matmul(zc[:, k, :], xj_sl,
                                                     wxj_s[:, layer, :],
                                                     start=False, stop=False)
                                    nc.tensor.matmul(zc[:, k, :], ea_sl,
                                                     wea_s[:, layer, :],
                                                     start=False, stop=True)

                                # zc holds [-z1 | z2] (z1-half weights
                                # sign-flipped on host).
                                # lnb = ln(1 + e^zc) = [softplus(-z1)|softplus(z2)]
                                # msg = softplus(z2) * sigmoid(z1)
                                #     = lnb_hi * exp(-lnb_lo)
                                ez = small_p.tile([128, 2, 256], f32, tag="ez")
                                nc.scalar.activation(ez[:, :npair, :],
                                                     zc[:, :npair, :], AF.Exp)
                                lnb = small_p.tile([128, 2, 256], f32, tag="lnb")
                                nc.scalar.activation(lnb[:, :npair, :],
                                                     ez[:, :npair, :], AF.Ln,
                                                     bias=ones_s[:])
                                sig = small_p.tile([128, 2, 128], f32, tag="sig")
                                nc.scalar.activation(sig[:, :npair, :],
                                                     lnb[:, :npair, 0:128],
                                                     AF.Exp, scale=-1.0)
                                msg = small_p.tile([128, 2, 128], bf, tag="msg")
                                nc.vector.tensor_tensor(
                                    out=msg[:, :npair, :],
                                    in0=sig[:, :npair, :],
                                    in1=lnb[:, :npair, 128:256],
                                    op=mybir.AluOpType.mult)

                                for k in range(npair):
                                    t = t0 + k
                                    nc.tensor.matmul(
                                        agg[:], sel_blk[:, t, :], msg[:, k, :],
                                        start=(t == 0), stop=(t == TPB - 1))

                            # ---- block epilogue: LN + residual + softplus --
                            xold = xio_p.tile([128, 128], f32, tag="xold")
                            eng2 = (nc.scalar, nc.sync)[blk % 2]
                            eng2.dma_start(
                                xold[:], xold_src[blk * 128:(blk + 1) * 128, :])

                            # row sums of agg and agg^2 via ACT accumulators
                            a_s = xio_p.tile([128, 128], f32, tag="as")
                            s1 = stats_p.tile([128, 1], f32, tag="s1")
                            nc.scalar.activation(a_s[:], agg[:], AF.Copy,
                                                 accum_out=s1[:])
                            sq_t = xio_p.tile([128, 128], f32, tag="sqt")
                            s2 = stats_p.tile([128, 1], f32, tag="s2")
                            nc.scalar.activation(sq_t[:], agg[:], AF.Square,
                                                 accum_out=s2[:])
                            mean = stats_p.tile([128, 1], f32, tag="mean")
                            nc.vector.tensor_scalar(
                                out=mean[:], in0=s1[:], scalar1=1.0 / 128.0,
                                scalar2=None, op0=mybir.AluOpType.mult)
                            var = stats_p.tile([128, 1], f32, tag="var")
                            # var = s2/128 - mean^2
                            msq = stats_p.tile([128, 1], f32, tag="msq")
                            nc.vector.tensor_mul(msq[:], mean[:], mean[:])
                            nc.vector.tensor_scalar(
                                out=var[:], in0=s2[:], scalar1=1.0 / 128.0,
                                scalar2=msq[:], op0=mybir.AluOpType.mult,
                                op1=mybir.AluOpType.subtract)
                            # rstd = exp(-0.5 * ln(var + eps))
                            lnv = stats_p.tile([128, 1], f32, tag="lnv")
                            nc.scalar.activation(lnv[:], var[:], AF.Ln,
                                                 bias=eps_s[:])
                            rstd = stats_p.tile([128, 1], f32, tag="rstd")
                            nc.scalar.activation(rstd[:], lnv[:], AF.Exp,
                                                 scale=-0.5)
                            nmr = stats_p.tile([128, 1], f32, tag="nmr")
                            nc.vector.tensor_scalar(
                                out=nmr[:], in0=mean[:], scalar1=-1.0,
                                scalar2=rstd[:], op0=mybir.AluOpType.mult,
                                op1=mybir.AluOpType.mult)

                            # xn = (agg - mean) * rstd  (ACT affine)
                            xn = xio_p.tile([128, 128], f32, tag="xn")
                            nc.scalar.activation(xn[:], a_s[:], AF.Identity,
                                                 scale=rstd[:], bias=nmr[:])
                            nc.vector.tensor_mul(xn[:], xn[:], g_s[:, layer, :])
                            nc.vector.tensor_add(xn[:], xn[:], b_s[:, layer, :])
                            nc.vector.tensor_add(xn[:], xn[:], xold[:])

                            # softplus(xn) = ln(1 + e^{xn})
                            exn = xio_p.tile([128, 128], f32, tag="exn")
                            nc.scalar.activation(exn[:], xn[:], AF.Exp)
                            xnew = xio_p.tile([128, 128], f32, tag="xnew")
                            nc.scalar.activation(xnew[:], exn[:], AF.Ln,
                                                 bias=ones_s[:])
                            if not last:
                                xbf = xio_p.tile([128, 128], bf, tag="xbf")
                                nc.scalar.activation(xbf[:], xnew[:], AF.Copy)
                                eng3 = (nc.sync, nc.scalar)[(blk + 1) % 2]
                                eng3.dma_start(
                                    xown[layer + 1][blk * 128:(blk + 1) * 128, :],
                                    xbf[:])
                                eng4 = (nc.scalar, nc.sync)[(blk + 1) % 2]
                                eng4.dma_start(
                                    xmast[layer][blk * 128:(blk + 1) * 128, :],
                                    xnew[:])
                                # feature-major copy for next layer's Y
                                xt_ps = selt_p.tile([128, 128], f32, tag="st")
                                nc.tensor.transpose(xt_ps[:], xnew[:], ident_s[:])
                                nc.vector.tensor_copy(
                                    fm_next[:, 0, blk * 128:(blk + 1) * 128],
                                    xt_ps[:])
                            else:
                                # pooled per-graph sums: gsum[g,:] += x[slot,:]
                                gsel = xio_p.tile([128, 256], f32, tag="gsel")
                                nc.vector.tensor_scalar(
                                    out=gsel[:], in0=io256_s[:],
                                    scalar1=gid_s[:, blk:blk + 1],
                                    scalar2=None, op0=mybir.AluOpType.is_equal)
                                nc.tensor.matmul(gsum0, gsel[:, 0:128],
                                                 xnew[:], start=(blk == 0),
                                                 stop=(blk == nblk - 1))
                                nc.tensor.matmul(gsum1, gsel[:, 128:256],
                                                 xnew[:], start=(blk == 0),
                                                 stop=(blk == nblk - 1))

                    # ---- exchange (layers 0,1): AllGather next x table ----
                    if not last:
                        nc.gpsimd.collective_compute(
                            "AllGather", mybir.AluOpType.bypass,
                            replica_groups=rg,
                            ins=[xown[layer + 1][:]], outs=[xall[layer + 1][:]])
                        xall3 = xall[layer + 1].rearrange(
                            "(r p) c -> p r c", p=128)
                        HR = ranks // 2
                        nc.sync.dma_start(tab_s[:, :HR, :], xall3[:, :HR, :])
                        nc.scalar.dma_start(tab_s[:, HR:, :], xall3[:, HR:, :])

                # ---- write pooled output ---------------------------------
                gsb = xio_p.tile([128, 256], f32, tag="gsb")
                nc.scalar.activation(gsb[:, 0:128], gsum0, AF.Copy)
                nc.scalar.activation(gsb[:, 128:256], gsum1, AF.Copy)
                nc.sync.dma_start(gfull_d[0:128, :], gsb[:, 0:128])
                nc.sync.dma_start(gfull_d[128:256, :], gsb[:, 128:256])
                nc.gpsimd.collective_compute(
                    "ReduceScatter", mybir.AluOpType.add, replica_groups=rg,
                    ins=[gfull_d], outs=[gpart_d])
                nc.sync.dma_start(gsum_d, gpart_d)
                if dbg:
                    nc.sync.dma_start(dx1_d, xown[1][:])
                    nc.sync.dma_start(dx2_d, xown[2][:])

    nc.compile()
    return nc


# --------------------------------------------------------------------------
# Host preprocessing
# --------------------------------------------------------------------------

def _softplus(x):
    return np.log1p(np.exp(-np.abs(x))) + np.maximum(x, 0.0)


def _snake_slots(n, n_bins):
    """Slot offsets (bin*128 + round) for n items dealt snake-wise, in the
    order of the sorted item list."""
    idx = np.arange(n)
    r = idx // n_bins
    k = idx % n_bins
    bins = np.where(r % 2 == 0, k, n_bins - 1 - k)
    return bins * 128 + r


def _wrap16(arr):
    # [n] int16 -> [16, n/16], idx i at (i%16, i//16)
    return np.ascontiguousarray(arr.reshape(-1, 16).T)


def preprocess(z, R, edge_index, batch, embedding, emb_w, emb_b, conv_w, conv_b,
               ln_g, ln_b, n_nodes=N_NODES, n_cores=N_CORES, nblk=NBLK,
               lo_slots=LO_SLOTS, edge_d=EDGE_D, cutoff=CUTOFF):
    blocks = n_cores * nblk
    slots = blocks * 128
    core_slots = nblk * 128
    lo_blocks = lo_slots // 128
    n_edges = edge_index.shape[1]
    src = np.asarray(edge_index[0], np.int64)
    dst = np.asarray(edge_index[1], np.int64)

    # edge distances on host (smearing runs on device)
    Rf = np.asarray(R, np.float32)
    d = np.linalg.norm(Rf[src] - Rf[dst], axis=-1)  # [E] f32

    # node permutation: jointly balance per-block lo- and hi-sourced
    # in-degrees (nodes with orig id < lo_slots must land in lo slots so the
    # int16 gather views stay valid).  Round-based LPT: each round assigns one
    # node to every block, heaviest nodes to the currently lightest blocks.
    islo_e = src < lo_slots
    a = np.bincount(dst[islo_e], minlength=n_nodes).astype(np.float64)
    b = np.bincount(dst[~islo_e], minlength=n_nodes).astype(np.float64)
    ca = max(a.sum() / blocks, 1.0)
    cb = max(b.sum() / blocks, 1.0)
    perm = np.full(n_nodes, -1, np.int64)

    def _pack(node_ids, nbins, slot0):
        w = a[node_ids] / ca + b[node_ids] / cb
        order = node_ids[np.argsort(-w, kind="stable")]
        la = np.zeros(nbins)
        lb = np.zeros(nbins)
        for r in range(order.size // nbins):
            chunk = order[r * nbins:(r + 1) * nbins]
            bins = np.argsort(la / ca + lb / cb, kind="stable")
            perm[chunk] = slot0 + bins * 128 + r
            la[bins] += a[chunk]
            lb[bins] += b[chunk]

    _pack(np.arange(lo_slots, dtype=np.int64), lo_blocks, 0)
    _pack(np.arange(lo_slots, n_nodes, dtype=np.int64), blocks - lo_blocks,
          lo_slots)
    # hi region has fewer nodes than slots: leftovers round-robin
    rem = np.flatnonzero(perm < 0)
    if rem.size:
        used = np.zeros(slots, bool)
        used[perm[perm >= 0]] = True
        free = np.flatnonzero(~used)
        perm[rem] = free[:rem.size]
    assert perm.min() >= 0

    es, ed = perm[src], perm[dst]
    blk = ed // 128

    lo_cnt = np.bincount(blk[islo_e], minlength=blocks)
    hi_cnt = np.bincount(blk[~islo_e], minlength=blocks)
    TL = int(-(-lo_cnt.max() // 128))
    TH = int(-(-hi_cnt.max() // 128))
    TPB = TL + TH
    S = nblk * TPB * 128

    # edge slot assignment: within block, lows first then highs
    key = blk * 2 + (~islo_e).astype(np.int64)
    eorder = np.argsort(key, kind="stable")
    ks = key[eorder]
    runstart = np.r_[0, np.flatnonzero(np.diff(ks)) + 1]
    runid = np.zeros(n_edges, np.int64)
    runid[runstart[1:]] = 1
    runid = np.cumsum(runid)
    pos = np.arange(n_edges) - runstart[runid]
    eb = ks // 2
    ehalf = ks % 2
    base = eb * TPB * 128 + ehalf * (TL * 128)
    eslot_g = base + pos
    core_of = eb // nblk
    eslot = eslot_g - core_of * (nblk * TPB * 128)

    ixlo = np.zeros((n_cores, nblk * TL * 128), np.int16)
    ixhi = np.zeros((n_cores, nblk * TH * 128), np.int16)
    dstv = np.full((n_cores, nblk * TPB, 128), -1.0, np.float32)
    u = np.zeros((n_cores, 1, S), np.float32)

    e_src = es[eorder]
    e_dst = ed[eorder]
    e_lo = ehalf == 0
    d_o = d[eorder]

    for c in range(n_cores):
        m = core_of == c
        sl = eslot[m]
        # xj
        mlo = m & e_lo
        mhi = m & ~e_lo
        slo_ = eslot[mlo]
        bb = slo_ // (TPB * 128)
        off = slo_ - bb * (TPB * 128)
        ixlo[c][bb * TL * 128 + off] = e_src[mlo].astype(np.int16)
        shi_ = eslot[mhi]
        bb = shi_ // (TPB * 128)
        off = shi_ - bb * (TPB * 128) - TL * 128
        ixhi[c][bb * TH * 128 + off] = (e_src[mhi] - lo_slots).astype(np.int16)
        # dst one-hot value, edge distances
        dstv[c].reshape(-1)[sl] = (e_dst[m] % 128).astype(np.float32)
        u[c, 0, sl] = d_o[m].astype(np.float32)

    # z tables (slot -> atom type; empty slots -> 100 which maps to a 0 row)
    zslot = np.full(slots, 100, np.int16)
    zslot[perm] = np.asarray(z, np.int16)
    # graph-id per slot (empty -> -1, excluded from pooling)
    gslot = np.full(slots, -1.0, np.float32)
    gslot[perm] = np.asarray(batch, np.float32)

    # embedding rows
    EWf = np.zeros((128, 128), np.float32)
    EWf[:100] = (np.asarray(embedding, np.float32)
                 @ np.asarray(emb_w, np.float32)
                 + np.asarray(emb_b, np.float32))
    EWb = EWf.astype(BF16)

    # conv weights; z1-half output columns sign-flipped so the device computes
    # [-z1 | z2] and can use exp/ln-only activations (one act table)
    cw = np.asarray(conv_w, np.float32).copy()
    cb = np.asarray(conv_b, np.float32).copy()
    cw[:, :, :128] *= -1.0
    cb[:, :128] *= -1.0
    wxi = np.ascontiguousarray(cw[:, :128, :].transpose(1, 0, 2)).astype(BF16)
    wxj = np.ascontiguousarray(cw[:, 128:256, :].transpose(1, 0, 2)).astype(BF16)
    wea = np.concatenate([cw[:, 256:, :], cb[:, None, :]], axis=1)
    wea = np.ascontiguousarray(wea.transpose(1, 0, 2)).astype(BF16)

    # LN gamma/beta rows
    lnr = np.concatenate(
        [np.concatenate([np.asarray(ln_g, np.float32)[l],
                         np.asarray(ln_b, np.float32)[l]])
         for l in range(cw.shape[0])])[None, :]

    # smearing: ea_k = exp(cfs_k * (d - offs_k)^2); cfs[100]=0 -> bias row 1
    offs = np.linspace(0.0, cutoff, edge_d, dtype=np.float32)
    coeff = np.float32(-0.5 / (offs[1] - offs[0]) ** 2)
    noffs = np.zeros((101, 1), np.float32)
    noffs[:edge_d, 0] = -offs
    cfs = np.zeros((101, 1), np.float32)
    cfs[:edge_d, 0] = coeff

    ior = np.arange(256, dtype=np.float32)[None, :]

    # ---- pack blobs ------------------------------------------------------
    uents, sents, BU, BS, BS8 = _layout(TL, TH, nblk, ranks=blocks,
                                        n_cores=n_cores)

    def pack(ents, arrays, nbytes):
        blob = np.zeros(nbytes // 2, np.int16)
        bv = blob.view(np.uint8)
        for name, (off, shape, dt_) in ents.items():
            a = np.ascontiguousarray(arrays[name])
            assert a.shape == tuple(shape) and a.dtype == np.dtype(dt_), \
                (name, a.shape, shape, a.dtype, dt_)
            bv[off:off + a.nbytes] = a.view(np.uint8).ravel()
        return blob

    sblob = pack(sents, {
        "ewb": EWb, "ewf": EWf,
        "wxi": wxi, "wxj": wxj, "wea": wea,
        "lnr": lnr, "ior": ior, "noffs": noffs, "cfs": cfs,
    }, BS)

    in_maps = []
    for c in range(n_cores):
        sl0 = c * core_slots
        uq = np.round(np.minimum(u[c], UQMAX) * (32767.0 / UQMAX)
                      ).astype(np.int16)
        ublob = pack(uents, {
            "u": uq,
            "ixlo": _wrap16(ixlo[c]),
            "ixhi": _wrap16(ixhi[c]),
            "zown": _wrap16(zslot[sl0:sl0 + core_slots]),
            "dstv": np.ascontiguousarray(
                dstv[c].transpose(1, 0)).astype(np.int8),
            "gid": np.ascontiguousarray(
                gslot[sl0:sl0 + core_slots].reshape(nblk, 128).T),
        }, BU)
        in_maps.append({
            "ublob": ublob,
            "sblob": sblob[c * BS8 // 2:(c + 1) * BS8 // 2],
        })
    return in_maps, TL, TH


# --------------------------------------------------------------------------
# execution: cached jitted SPMD runner (PJRT via bass2jax custom call)
# --------------------------------------------------------------------------

class _Results:
    """Minimal stand-in for BassKernelResults (test.py reads exec_time_ns)."""

    def __init__(self, results):
        self.results = results
        self.exec_time_ns = None


class _Runner:
    """Compile once, then run full numpy in_maps -> numpy outputs."""

    def __init__(self, nc, n_cores):
        import jax
        from jax.sharding import Mesh, PartitionSpec
        from jax.experimental.shard_map import shard_map
        from concourse import bass2jax

        bass2jax.install_neuronx_cc_hook()
        self.nc = nc
        self.n_cores = n_cores
        partition_name = (nc.partition_id_tensor.name
                          if nc.partition_id_tensor else None)
        in_names, out_names, out_avals, zero_outs = [], [], [], []
        for alloc in nc.m.functions[0].allocations:
            if not isinstance(alloc, mybir.MemoryLocationSet):
                continue
            name = alloc.memorylocations[0].name
            if alloc.kind == "ExternalInput":
                if name != partition_name:
                    in_names.append(name)
            elif alloc.kind == "ExternalOutput":
                shape = tuple(alloc.tensor_shape)
                dtype = mybir.dt.np(alloc.dtype)
                out_names.append(name)
                out_avals.append(jax.core.ShapedArray(shape, dtype))
                zero_outs.append(np.zeros((n_cores * shape[0], *shape[1:]),
                                          dtype))
        self.in_names = in_names
        self.out_names = out_names
        self.out_shapes = [tuple(a.shape) for a in out_avals]
        self.zero_outs = zero_outs
        n_params = len(in_names)
        all_in = in_names + out_names + (
            [partition_name] if partition_name else [])

        def _body(*args):
            operands = list(args)
            if partition_name is not None:
                operands.append(bass2jax.partition_id_tensor())
            outs = bass2jax._bass_exec_p.bind(
                *operands, out_avals=tuple(out_avals),
                in_names=tuple(all_in), out_names=tuple(out_names),
                lowering_input_output_aliases=(),
                sim_require_finite=True, sim_require_nnan=True, nc=nc)
            return tuple(outs)

        devs = jax.devices()[:n_cores]
        assert len(devs) == n_cores
        mesh = Mesh(np.asarray(devs), ("core",))
        n_outs = len(out_avals)
        self._fn = jax.jit(
            shard_map(_body, mesh=mesh,
                      in_specs=(PartitionSpec("core"),) * (n_params + n_outs),
                      out_specs=(PartitionSpec("core"),) * n_outs,
                      check_rep=False),
            donate_argnums=tuple(range(n_params, n_params + n_outs)),
            keep_unused=True)
    def run(self, in_maps):
        concat_in = [
            np.concatenate([np.asarray(m[n]) for m in in_maps], axis=0)
            for n in self.in_names]
        outs = self._fn(*concat_in, *self.zero_outs)
        n = self.n_cores
        return _Results([
            {name: np.asarray(outs[i]).reshape(n, *self.out_shapes[i])[c]
             for i, name in enumerate(self.out_names)}
            for c in range(n)])


def kernel(z, R, edge_index, batch, embedding, emb_w, emb_b, conv_w, conv_b,
           ln_g, ln_b, cfc_w, cfc_b, fc_w, fc_b, out_w, out_b):
    in_maps, TL, TH = preprocess(
        z, R, edge_index, batch, embedding, emb_w, emb_b, conv_w, conv_b,
        ln_g, ln_b)

    key = (TL, TH)
    if key not in _NC_CACHE:
        nc = build_nc(TL, TH)
        _NC_CACHE[key] = _Runner(nc, N_CORES)
    runner = _NC_CACHE[key]

    res = runner.run(in_maps)
    global LAST_RESULTS, LAST_RERUN_S, LAST_RUN
    LAST_RUN = (runner, in_maps)
    LAST_RESULTS = res
    if _os.environ.get("KERNEL_RERUN", "1") != "0":
        import time as _time
        t0 = _time.time()
        runner.run(in_maps)
        LAST_RERUN_S = _time.time() - t0

    gs = np.concatenate([res.results[c]["gsum"] for c in range(N_CORES)],
                        axis=0)  # [256, 128] fully-summed (reduce-scattered)

    batch = np.asarray(batch, np.int64)
    cnts = np.bincount(batch, minlength=N_GRAPHS).astype(np.float32)
    mol = gs / np.maximum(cnts, 1.0)[:, None]

    h = _softplus(mol @ np.asarray(cfc_w, np.float32) + np.asarray(cfc_b, np.float32))
    for l in range(np.asarray(fc_w).shape[0]):
        h = _softplus(h @ np.asarray(fc_w[l], np.float32)
                      + np.asarray(fc_b[l], np.float32))
    out = h @ np.asarray(out_w, np.float32) + np.asarray(out_b, np.float32)
    return out.astype(np.float32)
